# revision 2
# baseline (speedup 1.0000x reference)
"""3-layer GAT + graph pooling + MLP on 8 Trainium2 NeuronCores (Bass), v2.

Sharding: core c owns dst-nodes [c*NLOC, (c+1)*NLOC) and their in-edges.
All node tables are fp16; accumulation stays fp32 in PSUM.

Per layer:
  htab:   [h(192) | s_src(4) | s_dst(4) | pad] fp16 rows, 512B each, split in
          two half-tables (<=32768 rows each, int16 gather indices).  Layer 1
          computes it locally from the replicated features; layers 2-3
          AllGather per-core-computed own rows (h = x@Waug fused with the
          attention s columns), chunked in halves so the collective overlaps
          the previous layer's edge phase.
  edge:   two passes (one per half-table).  Per group of dst tiles:
          dma_gather 512B h-rows + 256B s_dst rows, w = exp(leaky(s)+shift)
          on ACT (fp16), scale h rows in place, one-hot via a single blocked
          is_equal, aggregate per 128-edge column on the TensorEngine:
          psum += onehot(dst)^T @ [w*h | w].  Pass 0 spills psum to an SBUF
          accumulator; pass 1 adds it back and runs post (softmax divide,
          bias, ELU) + transpose + the NEXT layer's own-row matmuls.
Pooling: as in v1 (unique-index scatter into graph-aligned slots, one-hot
sum pool, windowed max pool, small AllGather, replicated MLP).
All data-dependent structure (indices, one-hots) is INPUT DATA; the program
is static and identical across cores (SPMD).
"""

import sys
import numpy as np

sys.path.insert(0, "/opt/trn_rl_repo")

H, C = 4, 48
HC = H * C          # 192
NEG = 0.2
ESHIFT = -6.0       # global softmax shift: exp(leaky(u)+ESHIFT) fits fp16
BIGNEG = -2.0e30
MAXOFF = 1000.0     # max-pool offset: x3 = elu(...) >= -1, so x3+1000 > 0
ROW = 256           # htab row: h 192 | s_src 4 | s_dst 4 | pad (fp16)
SROW = 128          # stab row (fp16): s_dst 4 | pad  -> 256B
GROUP = 3           # dst tiles per gather call


def make_cfg(N=50000, E=800000, G=64, NC=8, FEAT=128, WIN=64):
    NLOC = N // NC
    assert NLOC * NC == N
    NTL = (NLOC + 127) // 128
    NSL = NTL * 128
    T0 = min((NTL + 1) // 2, max(1, int(NTL * 0.35)))
    T1 = NTL - T0
    assert T1 >= 1
    HALF0, HALF1 = T0 * 128, T1 * 128
    assert NC * HALF0 <= 32768 and NC * HALF1 <= 32768
    return dict(N=N, E=E, G=G, NC=NC, FEAT=FEAT, NLOC=NLOC, NSL=NSL, NTL=NTL,
                T0=T0, T1=T1, HALF0=HALF0, HALF1=HALF1, GSL=NC * NSL, WIN=WIN)


def _wrap_idx(idx):
    """SWDGE idx layout: element i -> [i % 16, i // 16], replicated to 128
    partitions (one copy per Q7 core)."""
    T = idx.shape[0]
    out = np.ascontiguousarray(idx.reshape(T // 16, 16).T).astype(np.int16)
    return np.tile(out, (8, 1))


def host_prep(cfg, adj, batch):
    N, G, NC = cfg["N"], cfg["G"], cfg["NC"]
    NLOC, NTL, WIN = cfg["NLOC"], cfg["NTL"], cfg["WIN"]
    NSL, T0, HALF0, HALF1 = cfg["NSL"], cfg["T0"], cfg["HALF0"], cfg["HALF1"]
    src = np.asarray(adj[0], dtype=np.int64)
    dst = np.asarray(adj[1], dtype=np.int64)
    batch = np.asarray(batch, dtype=np.int64)
    c_src = src // NLOC
    n_src = src % NLOC
    half_src = (n_src >= HALF0).astype(np.int64)
    row_src = np.where(half_src == 0, c_src * HALF0 + n_src,
                       c_src * HALF1 + (n_src - HALF0))
    counts_g = np.bincount(batch, minlength=G)

    # ---- edge grouping: (half, dst-tile) blocks, each padded %128, >=128
    blocks_all = []
    sizes = np.zeros((NC, 2, NTL), dtype=np.int64)
    for c in range(NC):
        lo = c * NLOC
        esel = np.nonzero((dst >= lo) & (dst < lo + NLOC))[0]
        dt_of = (dst[esel] - lo) // 128
        hf_of = half_src[esel]
        d = {}
        for h in range(2):
            for t in range(NTL):
                ee = esel[(hf_of == h) & (dt_of == t)]
                d[(h, t)] = ee
                sizes[c, h, t] = len(ee)
        blocks_all.append(d)
    bsz = np.zeros((2, NTL), dtype=np.int64)
    for h in range(2):
        for t in range(NTL):
            m = max(int(sizes[:, h, t].max()), 1)
            bsz[h, t] = -(-m // 128) * 128
    offs = np.zeros((2, NTL), dtype=np.int64)
    o = 0
    for h in range(2):
        for t in range(NTL):
            offs[h, t] = o
            o += bsz[h, t]
    TOT = int(o)

    # gather groups: (half, [tiles], i0, S) — every dst tile has a block in
    # each src half-table; greedy-pack tiles up to a column cap
    CAP = 30 * 128
    groups = []
    for h in range(2):
        tt, S = [], 0
        for t in range(NTL):
            b = int(bsz[h, t])
            if tt and S + b > CAP:
                groups.append((h, list(tt), int(offs[h, tt[0]]), S))
                tt, S = [], 0
            tt.append(t)
            S += b
        if tt:
            groups.append((h, list(tt), int(offs[h, tt[0]]), S))
    GC = max(S // 128 for (_, _, _, S) in groups)

    # ---- layer-3 graph-aligned slots (as v1)
    pad3_meta, pad3_tot = [], 0
    for c in range(NC):
        lo = c * NLOC
        b = batch[lo:lo + NLOC]
        gids, starts = np.unique(b, return_index=True)
        osort = np.argsort(starts)
        gids, starts = gids[osort], starts[osort]
        ends = np.append(starts[1:], NLOC)
        slots = np.empty(NLOC, dtype=np.int64)
        wg, fwin = [], []
        pos = 0
        for g, s, e in zip(gids, starts, ends):
            cnt = e - s
            slots[s:e] = pos + np.arange(cnt)
            nw = -(-cnt // WIN)
            wg += [int(g)] * nw
            fwin += [1] + [0] * (nw - 1)
            pos += nw * WIN
        pad3_meta.append((slots, wg, fwin))
        pad3_tot = max(pad3_tot, pos)
    PAD3 = -(-pad3_tot // 128) * 128
    NW, NT3 = PAD3 // WIN, PAD3 // 128
    assert NW <= 128
    # shared (min over cores) scatter lower bound per dst tile, for sliced
    # scatter APs so pooling window reductions can start before all tiles land
    lo_shared = np.full(NTL, 10**9, dtype=np.int64)
    for c in range(NC):
        slots3 = pad3_meta[c][0]
        for t in range(NTL):
            sl = slots3[t * 128:min((t + 1) * 128, NLOC)]
            if len(sl):
                lo_shared[t] = min(lo_shared[t], int(sl.min()))
    lo_shared = np.minimum.accumulate(lo_shared[::-1])[::-1]
    lo_shared[0] = 0
    rmap = np.zeros(NT3, dtype=np.int64)
    for r in range(NT3):
        sel = np.nonzero(lo_shared <= (r + 1) * 128 - 1)[0]
        rmap[r] = int(sel.max()) if len(sel) else 0
    cfg.update(TOT=TOT, bsz=bsz, offs=offs, groups=groups, GC=GC,
               PAD3=PAD3, NW=NW, NT3=NT3, lo_shared=lo_shared, rmap=rmap)

    gbase = {}
    for (h, tt, i0, S) in groups:
        for t in tt:
            gbase[(h, t)] = tt[0]

    data = []
    strides = [1, 2, 4, 8, 16, 32]
    for c in range(NC):
        lo = c * NLOC
        slots3, wg, fwin = pad3_meta[c]
        g1 = np.zeros(TOT, dtype=np.int64)
        g2 = np.zeros(TOT, dtype=np.int64)
        dstv = np.full(TOT, 999.0, dtype=np.float16)
        for h in range(2):
            for t in range(NTL):
                ee = blocks_all[c][(h, t)]
                i0 = int(offs[h, t])
                g1[i0:i0 + len(ee)] = row_src[ee]
                g2[i0:i0 + len(ee)] = (dst[ee] - lo) - gbase[(h, t)] * 128
                dstv[i0:i0 + len(ee)] = ((dst[ee] - lo) % 128).astype(np.float16)
        assert g1.min() >= 0 and g2.min() >= 0
        s3 = np.full(NSL, PAD3, dtype=np.int64)
        s3[:NLOC] = slots3
        for t in range(NTL):
            s3[t * 128:(t + 1) * 128] -= lo_shared[t]
        assert s3.min() >= 0
        wgp = np.full(NW, -1, dtype=np.int64)
        wgp[:len(wg)] = wg
        cmb = np.full((128, len(strides)), BIGNEG, dtype=np.float32)
        for k, s in enumerate(strides):
            for i in range(NW - s):
                if wgp[i] >= 0 and wgp[i] == wgp[i + s]:
                    cmb[i, k] = 0.0
        wplace = np.full(128, G, dtype=np.int64)
        for i in range(len(wg)):
            if fwin[i]:
                wplace[i] = wg[i]
        onehot = np.zeros((NTL, 128, G), dtype=np.float16)
        nn = np.arange(NLOC)
        onehot[nn // 128, nn % 128, batch[lo:lo + NLOC]] = 1.0
        data.append(dict(
            g1=_wrap_idx(g1),
            g2=_wrap_idx(g2),
            dstv=np.ascontiguousarray(
                dstv.reshape(TOT // 128, 128).T).astype(np.float16),
            s3=_wrap_idx(s3),
            cmb=cmb,
            wplace=_wrap_idx(wplace),
            onehot=onehot,
        ))
    inv_cnt = np.tile((1.0 / np.maximum(counts_g, 1.0))
                      .astype(np.float32)[None, :], (96, 1))
    return data, inv_cnt


def prep_float_inputs(cfg, inputs):
    NC, NLOC, NSL, FEAT = cfg["NC"], cfg["NLOC"], cfg["NSL"], cfg["FEAT"]
    HALF0, HALF1, GSL = cfg["HALF0"], cfg["HALF1"], cfg["GSL"]
    f = {}
    feat = np.asarray(inputs["features"], np.float32)
    fpad = np.zeros((NC, NSL, FEAT), np.float32)
    for c in range(NC):
        fpad[c, :NLOC] = feat[c * NLOC:(c + 1) * NLOC]
    h0 = fpad[:, :HALF0].reshape(NC * HALF0, FEAT)
    h1 = fpad[:, HALF0:].reshape(NC * HALF1, FEAT)
    f["featT"] = np.ascontiguousarray(
        np.concatenate([h0, h1], axis=0).T).astype(np.float16)
    for l in (1, 2, 3):
        W = np.asarray(inputs[f"W{l}"], np.float32)
        A = np.zeros((HC, 2 * H), np.float32)
        for h in range(H):
            A[h * C:(h + 1) * C, h] = np.asarray(inputs[f"a_src{l}"], np.float32)[h]
            A[h * C:(h + 1) * C, H + h] = np.asarray(inputs[f"a_dst{l}"], np.float32)[h]
        f[f"Waug{l}"] = np.concatenate([W, W @ A], axis=1).astype(np.float16)
        f[f"brep{l}"] = np.tile(np.asarray(inputs[f"b{l}"], np.float16)[None, :],
                                (128, 1))
    f["fc1_w"] = np.asarray(inputs["fc1_w"], np.float32)
    f["fc1_b"] = np.asarray(inputs["fc1_b"], np.float32).reshape(-1, 1)
    f["out_w"] = np.asarray(inputs["out_w"], np.float32)
    f["out_b"] = np.asarray(inputs["out_b"], np.float32).reshape(-1, 1)
    return f


def build_program(cfg):
    from concourse import bacc, bass, mybir, tile
    from concourse.masks import make_identity
    f32, f16, i16 = mybir.dt.float32, mybir.dt.float16, mybir.dt.int16
    AF, ALU = mybir.ActivationFunctionType, mybir.AluOpType
    G, NC, FEAT = cfg["G"], cfg["NC"], cfg["FEAT"]
    NLOC, NSL, NTL, GSL = cfg["NLOC"], cfg["NSL"], cfg["NTL"], cfg["GSL"]
    T0, T1, HALF0, HALF1 = cfg["T0"], cfg["T1"], cfg["HALF0"], cfg["HALF1"]
    TOT, PAD3 = cfg["TOT"], cfg["PAD3"]
    NW, NT3, WIN = cfg["NW"], cfg["NT3"], cfg["WIN"]
    NPW = 128 // WIN
    bsz, offs, groups, GC = cfg["bsz"], cfg["offs"], cfg["groups"], cfg["GC"]
    lo_shared, rmap = cfg["lo_shared"], cfg["rmap"]
    core_ids = list(range(NC))

    nc = bacc.Bacc(None, num_devices=NC, num_swdge_queues=2)

    featT = nc.declare_dram_parameter("featT", [FEAT, GSL], f16, False)
    featTo = nc.declare_dram_parameter("featTown", [FEAT, NSL], f16, False)
    Waug, brep = [], []
    for l in (1, 2, 3):
        Waug.append(nc.declare_dram_parameter(
            f"Waug{l}", [FEAT if l == 1 else HC, HC + 2 * H], f16, False))
        brep.append(nc.declare_dram_parameter(f"brep{l}", [128, HC], f16, False))
    fc1_w = nc.declare_dram_parameter("fc1_w", [2 * HC, 48], f32, False)
    fc1_b = nc.declare_dram_parameter("fc1_b", [48, 1], f32, False)
    out_w = nc.declare_dram_parameter("out_w", [48, 2], f32, False)
    out_b = nc.declare_dram_parameter("out_b", [2, 1], f32, False)
    inv_cnt = nc.declare_dram_parameter("inv_cnt", [96, G], f32, False)
    g1i = nc.declare_dram_parameter("g1", [128, TOT // 16], i16, False)
    g2i = nc.declare_dram_parameter("g2", [128, TOT // 16], i16, False)
    dstvi = nc.declare_dram_parameter("dstv", [128, TOT // 128], f16, False)
    s3i = nc.declare_dram_parameter("s3", [128, NSL // 16], i16, False)
    cmbi = nc.declare_dram_parameter("cmb", [128, 6], f32, False)
    wplacei = nc.declare_dram_parameter("wplace", [128, 8], i16, False)
    onehoti = nc.declare_dram_parameter("onehot", [NTL, 128, G], f16, False)
    yout = nc.declare_dram_parameter("y", [2, G], f32, True)

    htab = [nc.dram_tensor("htab0", [NC, HALF0, ROW], f16, addr_space="Shared"),
            nc.dram_tensor("htab1", [NC, HALF1, ROW], f16, addr_space="Shared")]
    own = [nc.dram_tensor("own0", [HALF0, ROW], f16),
           nc.dram_tensor("own1", [HALF1, ROW], f16)]
    stab = nc.dram_tensor("stab", [NSL, SROW], f16)
    padgrid = nc.dram_tensor("padgrid", [PAD3 + 128, HC], f32)
    maxgrid = nc.dram_tensor("maxgrid", [G + 1, HC], f32)
    poolsl = nc.dram_tensor("poolsl", [96, 4, G], f32)
    poolag = nc.dram_tensor("poolag", [NC, 96, 4, G], f32, addr_space="Shared")

    htabv = [htab[0][:].rearrange("c n e -> (c n) e"),
             htab[1][:].rearrange("c n e -> (c n) e")]

    with tile.TileContext(nc) as tc:
        with (
            tc.tile_pool(name="const", bufs=1) as constp,
            tc.tile_pool(name="wpool", bufs=1) as wpool,
            tc.tile_pool(name="lhs", bufs=3) as lhsp,
            tc.tile_pool(name="dense", bufs=3) as densep,
            tc.tile_pool(name="edge", bufs=3) as edgep,
            tc.tile_pool(name="edgec", bufs=2) as edgecp,
            tc.tile_pool(name="accp", bufs=1) as accp,
            tc.tile_pool(name="stagp", bufs=1) as stagp,
            tc.tile_pool(name="post", bufs=2) as postp,
            tc.tile_pool(name="xt", bufs=3) as xtp,
            tc.tile_pool(name="psum", bufs=2, space="PSUM") as psump,
            tc.tile_pool(name="psumP", bufs=1, space="PSUM") as psumPp,
            tc.tile_pool(name="small", bufs=1) as smallp,
        ):
            ident16 = constp.tile([128, 128], f16)
            make_identity(nc, ident16[:])
            identf = constp.tile([128, 128], f32)
            make_identity(nc, identf[:])
            iotai = constp.tile([128, 128], mybir.dt.int32)
            nc.gpsimd.iota(iotai[:], pattern=[[1, 128]], base=0,
                           channel_multiplier=0)
            iota16 = constp.tile([128, 128], f16)
            nc.vector.tensor_copy(iota16[:], iotai[:])
            eshift = constp.tile([128, 1], f32)
            nc.vector.memset(eshift[:], ESHIFT)
            zbias = constp.tile([128, 1], f32)
            nc.vector.memset(zbias[:], 0.0)

            wtA, wtB, bt = [], [], []
            for l in range(3):
                ka = FEAT if l == 0 else 96
                a = wpool.tile([ka, HC + 2 * H], f16, tag=f"wtA{l}")
                nc.sync.dma_start(a[:], Waug[l][:ka])
                wtA.append(a)
                if l == 0:
                    wtB.append(None)
                else:
                    b_ = wpool.tile([96, HC + 2 * H], f16, tag=f"wtB{l}")
                    nc.sync.dma_start(b_[:], Waug[l][96:])
                    wtB.append(b_)
                bb = wpool.tile([128, HC], f16, tag=f"bt{l}")
                nc.sync.dma_start(bb[:], brep[l][:])
                bt.append(bb)
            idxt = {}
            for nm, dram, w_ in (("g1", g1i, TOT // 16), ("g2", g2i, TOT // 16),
                                 ("s3", s3i, NSL // 16)):
                t = wpool.tile([128, w_], i16, tag=f"ix{nm}")
                nc.sync.dma_start(t[:], dram[:])
                idxt[nm] = t
            dstvt = wpool.tile([128, TOT // 128], f16, tag="dstv")
            nc.sync.dma_start(dstvt[:], dstvi[:])
            cmbt = wpool.tile([128, 6], f32, tag="cmb")
            nc.sync.dma_start(cmbt[:], cmbi[:])
            wplt = wpool.tile([128, 8], i16, tag="wpl")
            nc.sync.dma_start(wplt[:], wplacei[:])
            invt = wpool.tile([96, G], f32, tag="inv")
            nc.sync.dma_start(invt[:], inv_cnt[:])
            fc1wt = []
            for k in range(4):
                t = wpool.tile([96, 48], f32, tag=f"fc1{k}")
                nc.sync.dma_start(t[:], fc1_w[k * 96:(k + 1) * 96])
                fc1wt.append(t)
            fc1bt = wpool.tile([48, 1], f32, tag="fc1b")
            nc.sync.dma_start(fc1bt[:], fc1_b[:])
            outwt = wpool.tile([48, 2], f32, tag="outw")
            nc.sync.dma_start(outwt[:], out_w[:])
            outbt = wpool.tile([2, 1], f32, tag="outb")
            nc.sync.dma_start(outbt[:], out_b[:])

            acc = accp.tile([128, NTL, HC + H], f16, tag="acc")
            stag = [stagp.tile([128, T0, ROW], f16, tag="stag0", name="stag0"),
                    stagp.tile([128, T1, ROW], f16, tag="stag1", name="stag1")]
            stabst = stagp.tile([128, NTL, H], f16, tag="stabst")

            zt = constp.tile([128, 4, HC], f32, tag="zt")
            nc.vector.memset(zt[:], 0.0)
            r0 = 0
            while r0 < PAD3 + 128:
                rr = min(512, PAD3 + 128 - r0)
                nc.sync.dma_start(
                    padgrid[r0:r0 + rr].rearrange("(p a) c -> p (a c)", p=128),
                    zt[:, :rr // 128].rearrange("p a c -> p (a c)"))
                r0 += rr

            # ---- layer-1 stab (own s_dst rows from the per-core features)
            for t in range(NTL):
                lhs = lhsp.tile([FEAT, 128], f16, tag="slhs")
                nc.sync.dma_start(lhs[:], featTo[:, t * 128:(t + 1) * 128])
                ps = psump.tile([128, H], f32, tag="dps")
                nc.tensor.matmul(ps[:], lhs[:], wtA[0][:, HC + H:],
                                 start=True, stop=True)
                nc.scalar.activation(stabst[:, t], ps[:], AF.Copy)
            nc.sync.dma_start(
                stab[:].rearrange("(t p) c -> p t c", p=128)[:, :, :H],
                stabst[:])

            # ---- layer-1 dense-full: every core computes the whole htab
            BT = 4
            for half in (0, 1):
                ntile = NC * (T0 if half == 0 else T1)
                hv = htabv[half]
                for b0 in range(0, ntile, BT):
                    nb = min(BT, ntile - b0)
                    lhs = lhsp.tile([FEAT, BT * 128], f16, tag="dlhs")
                    nc.sync.dma_start(
                        lhs[:, :nb * 128],
                        featT[:, (half * NC * HALF0) + b0 * 128:
                              (half * NC * HALF0) + (b0 + nb) * 128])
                    hrow = densep.tile([128, BT, ROW], f16, tag="hrow")
                    for j in range(nb):
                        ps = psump.tile([128, HC + 2 * H], f32, tag="dps")
                        nc.tensor.matmul(ps[:], lhs[:, j * 128:(j + 1) * 128],
                                         wtA[0][:], start=True, stop=True)
                        nc.vector.tensor_copy(hrow[:, j, :HC + 2 * H], ps[:])
                    nc.scalar.dma_start(
                        hv[b0 * 128:(b0 + nb) * 128]
                        .rearrange("(j p) e -> p j e", p=128),
                        hrow[:, :nb])

            def post_tile(l, t, tmp):
                """tmp: SBUF fp32 [128, HC+H] aggregated messages + denoms."""
                den = postp.tile([128, H], f32, tag="pden")
                nc.vector.tensor_scalar(den[:], tmp[:, HC:], 1e-16, None,
                                        ALU.max)
                nc.vector.reciprocal(den[:], den[:])
                y16 = postp.tile([128, HC], f16, tag="py")
                for h in range(H):
                    nc.vector.tensor_scalar(
                        y16[:, h * C:(h + 1) * C], tmp[:, h * C:(h + 1) * C],
                        den[:, h:h + 1], None, ALU.mult)
                nc.vector.scalar_tensor_tensor(
                    y16[:], y16[:], 1.0, bt[l][:], ALU.bypass, ALU.add)
                e16 = postp.tile([128, HC], f16, tag="pe")
                nc.vector.tensor_scalar(e16[:], y16[:], 0.0, None, ALU.min)
                nc.scalar.activation(e16[:], e16[:], AF.Exp)
                nc.vector.scalar_tensor_tensor(
                    y16[:], e16[:], -1.0, y16[:], ALU.add, ALU.max)
                return y16

            def own_rows(l, t, y16):
                """Transpose y16 and compute next layer's own htab/stab rows."""
                xt = xtp.tile([96, 2, 128], f16, tag="xt")
                for blk in range(2):
                    pt = psump.tile([96, 128], f16, tag="tps")
                    nc.tensor.transpose(
                        pt[:], y16[:, blk * 96:(blk + 1) * 96], ident16[:])
                    nc.scalar.activation(xt[:, blk], pt[:], AF.Copy)
                ps2 = psump.tile([128, HC + 2 * H], f32, tag="dps")
                nc.tensor.matmul(ps2[:], xt[:, 0], wtA[l + 1][:],
                                 start=True, stop=False)
                nc.tensor.matmul(ps2[:], xt[:, 1], wtB[l + 1][:],
                                 start=False, stop=True)
                half = 0 if t < T0 else 1
                tl = t if t < T0 else t - T0
                nc.scalar.activation(stag[half][:, tl, :HC + 2 * H], ps2[:],
                                     AF.Copy)
                nc.scalar.activation(stabst[:, t], ps2[:, HC + H:], AF.Copy)
                if t == T0 - 1:
                    nc.sync.dma_start(
                        own[0][:].rearrange("(t p) e -> p t e", p=128),
                        stag[0][:])
                    nc.gpsimd.collective_compute(
                        "AllGather", ALU.bypass, replica_groups=[core_ids],
                        ins=[own[0][:]], outs=[htab[0][:]])
                if t == NTL - 1:
                    nc.sync.dma_start(
                        stab[:].rearrange("(t p) c -> p t c", p=128)[:, :, :H],
                        stabst[:])
                    nc.sync.dma_start(
                        own[1][:].rearrange("(t p) e -> p t e", p=128),
                        stag[1][:])

            wmax = stagp.tile([96, 2, NW], f32, tag="wmax")

            def wmax_tile(r):
                rows = postp.tile([128, HC], f32, tag="prow3")
                nc.sync.dma_start(rows[:], padgrid[r * 128:(r + 1) * 128])
                for blk in range(2):
                    pt = psump.tile([96, 128], f32, tag="tps")
                    nc.tensor.transpose(
                        pt[:], rows[:, blk * 96:(blk + 1) * 96], identf[:])
                    nc.vector.tensor_reduce(
                        wmax[:, blk, r * NPW:(r + 1) * NPW],
                        pt[:].rearrange("p (w q) -> p w q", q=WIN),
                        mybir.AxisListType.X, ALU.max)

            ohall = wpool.tile([128, NTL, G], f16, tag="ohall")
            nc.sync.dma_start(
                ohall[:], onehoti[:].rearrange("t p g -> p t g"))

            def pool_tile(t, y16, sump):
                for blk in range(2):
                    nc.tensor.matmul(
                        sump[blk][:], y16[:, blk * 96:(blk + 1) * 96],
                        ohall[:, t], start=(t == 0), stop=(t == NTL - 1))
                yo = postp.tile([128, HC], f32, tag="pyo")
                nc.vector.tensor_scalar(yo[:], y16[:], MAXOFF, None, ALU.add)
                nc.gpsimd.dma_scatter_add(
                    padgrid[int(lo_shared[t]):, :],
                    yo[:].rearrange("p (a c) -> p a c", a=1),
                    idxt["s3"][:, t * 8:(t + 1) * 8], 128, 128, HC,
                    single_packet=False)

            def edge_pass(l, half, sump):
                for (h, tt, i0, S) in groups:
                        if h != half:
                            continue
                        cols = S // 128
                        big = edgep.tile([128, GC, ROW], f16, tag="big")
                        nc.gpsimd.dma_gather(
                            big[:, :cols], htabv[half],
                            idxt["g1"][:, i0 // 16:(i0 + S) // 16], S, S, ROW,
                            single_packet=False, queue_num=0)
                        sd = edgep.tile([128, GC, SROW], f16, tag="sd")
                        sbase = tt[0] * 128
                        nc.gpsimd.dma_gather(
                            sd[:, :cols], stab[sbase:sbase + len(tt) * 128],
                            idxt["g2"][:, i0 // 16:(i0 + S) // 16], S, S, SROW,
                            single_packet=False, queue_num=1)
                        u = edgecp.tile([128, GC, H], f32, tag="u")
                        nc.vector.scalar_tensor_tensor(
                            u[:, :cols], big[:, :cols, HC:HC + H], 1.0,
                            sd[:, :cols, :H], ALU.bypass, ALU.add)
                        um = edgecp.tile([128, GC, H], f32, tag="um")
                        nc.vector.scalar_tensor_tensor(
                            um[:, :cols], u[:, :cols], NEG, u[:, :cols],
                            ALU.mult, ALU.max)
                        w4 = edgecp.tile([128, GC, H], f16, tag="w4")
                        nc.scalar.activation(w4[:, :cols], um[:, :cols],
                                             AF.Exp, bias=eshift[:])
                        WSUB = 8
                        for c0 in range(0, cols, WSUB):
                            cc = min(WSUB, cols - c0)
                            wfull = edgecp.tile([128, WSUB, HC], f16,
                                               tag="wfull")
                            nc.scalar.activation(
                                wfull[:, :cc].rearrange(
                                    "p c (h k) -> p c h k", h=H),
                                w4[:, c0:c0 + cc, :, None]
                                .to_broadcast([128, cc, H, C]), AF.Copy)
                            nc.vector.tensor_tensor(
                                big[:, c0:c0 + cc, :HC],
                                big[:, c0:c0 + cc, :HC],
                                wfull[:, :cc], ALU.mult)
                        nc.vector.tensor_copy(big[:, :cols, HC:HC + H],
                                              w4[:, :cols])
                        oh = edgecp.tile([128, GC, 128], f16, tag="oh")
                        nc.vector.tensor_tensor(
                            oh[:, :cols],
                            iota16[:, None, :].to_broadcast([128, cols, 128]),
                            dstvt[:, i0 // 128:i0 // 128 + cols, None]
                            .to_broadcast([128, cols, 128]),
                            ALU.is_equal)
                        q0 = 0
                        for t in tt:
                            ncq = int(bsz[half, t]) // 128
                            ps = psump.tile([128, HC + H], f32, tag="agg")
                            for q in range(ncq):
                                nc.tensor.matmul(
                                    ps[:], oh[:, q0 + q],
                                    big[:, q0 + q, :HC + H],
                                    start=(q == 0), stop=(q == ncq - 1))
                            q0 += ncq
                            if half == 0:
                                nc.scalar.activation(acc[:, t], ps[:], AF.Copy)
                            else:
                                tmp = postp.tile([128, HC + H], f32, tag="tmp")
                                nc.vector.scalar_tensor_tensor(
                                    tmp[:], ps[:], 1.0, acc[:, t],
                                    ALU.bypass, ALU.add)
                                y16 = post_tile(l, t, tmp)
                                if l < 2:
                                    own_rows(l, t, y16)
                                else:
                                    pool_tile(t, y16, sump)

            sump0 = psumPp.tile([96, G], f32, tag="sum0")
            sump1 = psumPp.tile([96, G], f32, tag="sum1")
            sump = [sump0, sump1]
            edge_pass(0, 0, None)
            for l in range(3):
                edge_pass(l, 1, sump if l == 2 else None)
                if l < 2:
                    edge_pass(l + 1, 0, None)
                    nc.gpsimd.collective_compute(
                        "AllGather", ALU.bypass, replica_groups=[core_ids],
                        ins=[own[1][:]], outs=[htab[1][:]])

            # ---- pooling epilogue (fp32, as v1)
            for r in range(NT3):
                wmax_tile(r)
            wrow = smallp.tile([128, HC], f32, tag="wrow")
            for blk in range(2):
                pt2 = psump.tile([128, 96], f32, tag="tps")
                nc.tensor.transpose(pt2[:NW], wmax[:, blk],
                                    identf[:96, :96])
                nc.vector.tensor_copy(
                    wrow[:NW, blk * 96:(blk + 1) * 96], pt2[:NW])
            for ki, s in enumerate([1, 2, 4, 8, 16, 32]):
                if s >= NW:
                    break
                sh = smallp.tile([128, HC], f32, tag="wsh")
                nc.sync.dma_start(sh[:NW - s], wrow[s:NW])
                nc.vector.tensor_scalar(sh[:NW - s], sh[:NW - s],
                                        cmbt[:NW - s, ki:ki + 1],
                                        None, ALU.add)
                nc.vector.tensor_max(wrow[:NW - s], wrow[:NW - s],
                                     sh[:NW - s])
            zg = smallp.tile([G + 1, HC], f32, tag="zg")
            nc.vector.memset(zg[:], 0.0)
            nc.sync.dma_start(maxgrid[:], zg[:])
            nc.gpsimd.dma_scatter_add(
                maxgrid[:], wrow[:].rearrange("p (a c) -> p a c", a=1),
                wplt[:], 128, 128, HC, single_packet=False)
            mg = smallp.tile([G, HC], f32, tag="mg")
            nc.sync.dma_start(mg[:], maxgrid[:G])
            pp = smallp.tile([96, 4, G], f32, tag="pp")
            for blk in range(2):
                nc.vector.tensor_copy(pp[:, blk], sump0[:] if blk == 0
                                      else sump1[:])
                pt3 = psump.tile([96, G], f32, tag="tps")
                nc.tensor.transpose(
                    pt3[:], mg[:, blk * 96:(blk + 1) * 96],
                    identf[:G, :G])
                nc.vector.tensor_copy(pp[:, 2 + blk], pt3[:])
            nc.sync.dma_start(poolsl[:], pp[:])
            nc.gpsimd.collective_compute(
                "AllGather", ALU.bypass,
                replica_groups=[core_ids],
                ins=[poolsl[:]], outs=[poolag[:]])
            agg2 = smallp.tile([96, 4, G], f32, tag="agg2")
            for c_ in range(NC):
                at = smallp.tile([96, 4, G], f32, tag="agt")
                nc.sync.dma_start(at[:], poolag[c_])
                if c_ == 0:
                    nc.vector.tensor_copy(agg2[:], at[:])
                else:
                    nc.vector.tensor_add(agg2[:, :2], agg2[:, :2],
                                         at[:, :2])
                    nc.vector.tensor_max(agg2[:, 2:], agg2[:, 2:],
                                         at[:, 2:])
            for blk in range(2):
                nc.vector.tensor_mul(agg2[:, blk], agg2[:, blk], invt[:])
                nc.vector.tensor_scalar(agg2[:, 2 + blk],
                                        agg2[:, 2 + blk],
                                        -MAXOFF, None, ALU.add)
            zp = psump.tile([48, G], f32, tag="tps")
            for k in range(4):
                nc.tensor.matmul(zp[:], fc1wt[k], agg2[:, k],
                                 start=(k == 0), stop=(k == 3))
            z = smallp.tile([48, G], f32, tag="z")
            nc.vector.tensor_scalar(z[:], zp[:], fc1bt[:], None,
                                    ALU.add)
            e2 = smallp.tile([48, G], f32, tag="e2")
            nc.vector.tensor_scalar(e2[:], z[:], 0.0, None, ALU.min)
            nc.scalar.activation(e2[:], e2[:], AF.Exp)
            nc.vector.tensor_scalar(e2[:], e2[:], -1.0, None, ALU.add)
            nc.vector.tensor_max(z[:], z[:], e2[:])
            yp = psump.tile([2, G], f32, tag="tps")
            nc.tensor.matmul(yp[:], outwt[:], z[:], start=True,
                             stop=True)
            yf = smallp.tile([2, G], f32, tag="yf")
            nc.vector.tensor_scalar(yf[:], yp[:], outbt[:], None,
                                    ALU.add)
            nc.sync.dma_start(yout[:], yf[:])
    nc.finalize()
    return nc


def run(inputs, cfg, **run_kw):
    data, inv_cnt = host_prep(cfg, inputs["adj"], inputs["batch"])
    fl = prep_float_inputs(cfg, inputs)
    NC, NLOC, NSL = cfg["NC"], cfg["NLOC"], cfg["NSL"]
    in_maps = []
    for c in range(NC):
        m = dict(fl)
        m["inv_cnt"] = inv_cnt
        fto = np.zeros((cfg["FEAT"], NSL), np.float16)
        fto[:, :NLOC] = np.asarray(inputs["features"],
                                   np.float32)[c * NLOC:(c + 1) * NLOC].T
        m["featTown"] = fto
        m.update({k: data[c][k] for k in
                  ("g1", "g2", "dstv", "s3", "cmb", "wplace", "onehot")})
        in_maps.append(m)
    nc = build_program(cfg)
    from concourse.bass_utils import run_bass_kernel_spmd
    res = run_bass_kernel_spmd(nc, in_maps, list(range(NC)), **run_kw)
    y = np.asarray(res.results[0]["y"])
    return y.T.copy(), res


def kernel(**inputs):
    y, _ = run(inputs, make_cfg())
    return y


# revision 3
# speedup vs baseline: 1.0204x; 1.0204x over previous
"""3-layer GAT + graph pooling + MLP on 8 Trainium2 NeuronCores (Bass), v2.

Sharding: core c owns dst-nodes [c*NLOC, (c+1)*NLOC) and their in-edges.
All node tables are fp16; accumulation stays fp32 in PSUM.

Per layer:
  htab:   [h(192) | s_src(4) | s_dst(4) | pad] fp16 rows, 512B each, split in
          two half-tables (<=32768 rows each, int16 gather indices).  Layer 1
          computes it locally from the replicated features; layers 2-3
          AllGather per-core-computed own rows (h = x@Waug fused with the
          attention s columns), chunked in halves so the collective overlaps
          the previous layer's edge phase.
  edge:   two passes (one per half-table).  Per group of dst tiles:
          dma_gather 512B h-rows + 256B s_dst rows, w = exp(leaky(s)+shift)
          on ACT (fp16), scale h rows in place, one-hot via a single blocked
          is_equal, aggregate per 128-edge column on the TensorEngine:
          psum += onehot(dst)^T @ [w*h | w].  Pass 0 spills psum to an SBUF
          accumulator; pass 1 adds it back and runs post (softmax divide,
          bias, ELU) + transpose + the NEXT layer's own-row matmuls.
Pooling: as in v1 (unique-index scatter into graph-aligned slots, one-hot
sum pool, windowed max pool, small AllGather, replicated MLP).
All data-dependent structure (indices, one-hots) is INPUT DATA; the program
is static and identical across cores (SPMD).
"""

import sys
import numpy as np

sys.path.insert(0, "/opt/trn_rl_repo")

H, C = 4, 48
HC = H * C          # 192
NEG = 0.2
ESHIFT = -6.0       # global softmax shift: exp(leaky(u)+ESHIFT) fits fp16
BIGNEG = -2.0e30
MAXOFF = 1000.0     # max-pool offset: x3 = elu(...) >= -1, so x3+1000 > 0
ROW = 256           # htab row: h 192 | s_src 4 | s_dst 4 | pad (fp16)
SROW = 128          # stab row (fp16): s_dst 4 | pad  -> 256B
GROUP = 3           # dst tiles per gather call


def make_cfg(N=50000, E=800000, G=64, NC=8, FEAT=128, WIN=64):
    NLOC = N // NC
    assert NLOC * NC == N
    NTL = (NLOC + 127) // 128
    NSL = NTL * 128
    T0 = min((NTL + 1) // 2, max(1, int(NTL * 0.35)))
    T1 = NTL - T0
    assert T1 >= 1
    HALF0, HALF1 = T0 * 128, T1 * 128
    assert NC * HALF0 <= 32768 and NC * HALF1 <= 32768
    return dict(N=N, E=E, G=G, NC=NC, FEAT=FEAT, NLOC=NLOC, NSL=NSL, NTL=NTL,
                T0=T0, T1=T1, HALF0=HALF0, HALF1=HALF1, GSL=NC * NSL, WIN=WIN)


def _wrap_idx(idx):
    """SWDGE idx layout: element i -> [i % 16, i // 16], replicated to 128
    partitions (one copy per Q7 core)."""
    T = idx.shape[0]
    out = np.ascontiguousarray(idx.reshape(T // 16, 16).T).astype(np.int16)
    return np.tile(out, (8, 1))


def host_prep(cfg, adj, batch):
    N, G, NC = cfg["N"], cfg["G"], cfg["NC"]
    NLOC, NTL, WIN = cfg["NLOC"], cfg["NTL"], cfg["WIN"]
    NSL, T0, HALF0, HALF1 = cfg["NSL"], cfg["T0"], cfg["HALF0"], cfg["HALF1"]
    src = np.asarray(adj[0], dtype=np.int64)
    dst = np.asarray(adj[1], dtype=np.int64)
    batch = np.asarray(batch, dtype=np.int64)
    c_src = src // NLOC
    n_src = src % NLOC
    half_src = (n_src >= HALF0).astype(np.int64)
    row_src = np.where(half_src == 0, c_src * HALF0 + n_src,
                       c_src * HALF1 + (n_src - HALF0))
    counts_g = np.bincount(batch, minlength=G)

    # ---- edge grouping: (half, dst-tile) blocks, each padded %128, >=128
    blocks_all = []
    sizes = np.zeros((NC, 2, NTL), dtype=np.int64)
    for c in range(NC):
        lo = c * NLOC
        esel = np.nonzero((dst >= lo) & (dst < lo + NLOC))[0]
        dt_of = (dst[esel] - lo) // 128
        hf_of = half_src[esel]
        d = {}
        for h in range(2):
            for t in range(NTL):
                ee = esel[(hf_of == h) & (dt_of == t)]
                d[(h, t)] = ee
                sizes[c, h, t] = len(ee)
        blocks_all.append(d)
    bsz = np.zeros((2, NTL), dtype=np.int64)
    for h in range(2):
        for t in range(NTL):
            m = max(int(sizes[:, h, t].max()), 1)
            bsz[h, t] = -(-m // 128) * 128
    offs = np.zeros((2, NTL), dtype=np.int64)
    o = 0
    for h in range(2):
        for t in range(NTL):
            offs[h, t] = o
            o += bsz[h, t]
    TOT = int(o)

    # gather groups: (half, [tiles], i0, S) — every dst tile has a block in
    # each src half-table; greedy-pack tiles up to a column cap
    CAP = 30 * 128
    groups = []
    for h in range(2):
        tt, S = [], 0
        for t in range(NTL):
            b = int(bsz[h, t])
            if tt and S + b > CAP:
                groups.append((h, list(tt), int(offs[h, tt[0]]), S))
                tt, S = [], 0
            tt.append(t)
            S += b
        if tt:
            groups.append((h, list(tt), int(offs[h, tt[0]]), S))
    GC = max(S // 128 for (_, _, _, S) in groups)

    # ---- layer-3 graph-aligned slots (as v1)
    pad3_meta, pad3_tot = [], 0
    for c in range(NC):
        lo = c * NLOC
        b = batch[lo:lo + NLOC]
        gids, starts = np.unique(b, return_index=True)
        osort = np.argsort(starts)
        gids, starts = gids[osort], starts[osort]
        ends = np.append(starts[1:], NLOC)
        slots = np.empty(NLOC, dtype=np.int64)
        wg, fwin = [], []
        pos = 0
        for g, s, e in zip(gids, starts, ends):
            cnt = e - s
            slots[s:e] = pos + np.arange(cnt)
            nw = -(-cnt // WIN)
            wg += [int(g)] * nw
            fwin += [1] + [0] * (nw - 1)
            pos += nw * WIN
        pad3_meta.append((slots, wg, fwin))
        pad3_tot = max(pad3_tot, pos)
    PAD3 = -(-pad3_tot // 128) * 128
    NW, NT3 = PAD3 // WIN, PAD3 // 128
    assert NW <= 128
    # shared (min over cores) scatter lower bound per dst tile, for sliced
    # scatter APs so pooling window reductions can start before all tiles land
    lo_shared = np.full(NTL, 10**9, dtype=np.int64)
    for c in range(NC):
        slots3 = pad3_meta[c][0]
        for t in range(NTL):
            sl = slots3[t * 128:min((t + 1) * 128, NLOC)]
            if len(sl):
                lo_shared[t] = min(lo_shared[t], int(sl.min()))
    lo_shared = np.minimum.accumulate(lo_shared[::-1])[::-1]
    lo_shared[0] = 0
    rmap = np.zeros(NT3, dtype=np.int64)
    for r in range(NT3):
        sel = np.nonzero(lo_shared <= (r + 1) * 128 - 1)[0]
        rmap[r] = int(sel.max()) if len(sel) else 0
    cfg.update(TOT=TOT, bsz=bsz, offs=offs, groups=groups, GC=GC,
               PAD3=PAD3, NW=NW, NT3=NT3, lo_shared=lo_shared, rmap=rmap)

    gbase = {}
    for (h, tt, i0, S) in groups:
        for t in tt:
            gbase[(h, t)] = tt[0]

    data = []
    strides = [1, 2, 4, 8, 16, 32]
    for c in range(NC):
        lo = c * NLOC
        slots3, wg, fwin = pad3_meta[c]
        g1 = np.zeros(TOT, dtype=np.int64)
        g2 = np.zeros(TOT, dtype=np.int64)
        dstv = np.full(TOT, 999.0, dtype=np.float16)
        for h in range(2):
            for t in range(NTL):
                ee = blocks_all[c][(h, t)]
                i0 = int(offs[h, t])
                g1[i0:i0 + len(ee)] = row_src[ee]
                g2[i0:i0 + len(ee)] = (dst[ee] - lo) - gbase[(h, t)] * 128
                dstv[i0:i0 + len(ee)] = ((dst[ee] - lo) % 128).astype(np.float16)
        assert g1.min() >= 0 and g2.min() >= 0
        s3 = np.full(NSL, PAD3, dtype=np.int64)
        s3[:NLOC] = slots3
        for t in range(NTL):
            s3[t * 128:(t + 1) * 128] -= lo_shared[t]
        assert s3.min() >= 0
        wgp = np.full(NW, -1, dtype=np.int64)
        wgp[:len(wg)] = wg
        cmb = np.full((128, len(strides)), BIGNEG, dtype=np.float32)
        for k, s in enumerate(strides):
            for i in range(NW - s):
                if wgp[i] >= 0 and wgp[i] == wgp[i + s]:
                    cmb[i, k] = 0.0
        wplace = np.full(128, G, dtype=np.int64)
        for i in range(len(wg)):
            if fwin[i]:
                wplace[i] = wg[i]
        onehot = np.zeros((NTL, 128, G), dtype=np.float16)
        nn = np.arange(NLOC)
        onehot[nn // 128, nn % 128, batch[lo:lo + NLOC]] = 1.0
        data.append(dict(
            g1=_wrap_idx(g1),
            g2=_wrap_idx(g2),
            dstv=np.ascontiguousarray(
                dstv.reshape(TOT // 128, 128).T).astype(np.float16),
            s3=_wrap_idx(s3),
            cmb=cmb,
            wplace=_wrap_idx(wplace),
            onehot=onehot,
        ))
    inv_cnt = np.tile((1.0 / np.maximum(counts_g, 1.0))
                      .astype(np.float32)[None, :], (96, 1))
    return data, inv_cnt


def prep_float_inputs(cfg, inputs):
    NC, NLOC, NSL, FEAT = cfg["NC"], cfg["NLOC"], cfg["NSL"], cfg["FEAT"]
    HALF0, HALF1, GSL = cfg["HALF0"], cfg["HALF1"], cfg["GSL"]
    f = {}
    feat = np.asarray(inputs["features"], np.float32)
    fpad = np.zeros((NC, NSL, FEAT), np.float32)
    for c in range(NC):
        fpad[c, :NLOC] = feat[c * NLOC:(c + 1) * NLOC]
    h0 = fpad[:, :HALF0].reshape(NC * HALF0, FEAT)
    h1 = fpad[:, HALF0:].reshape(NC * HALF1, FEAT)
    f["featT"] = np.ascontiguousarray(
        np.concatenate([h0, h1], axis=0).T).astype(np.float16)
    for l in (1, 2, 3):
        W = np.asarray(inputs[f"W{l}"], np.float32)
        A = np.zeros((HC, 2 * H), np.float32)
        for h in range(H):
            A[h * C:(h + 1) * C, h] = np.asarray(inputs[f"a_src{l}"], np.float32)[h]
            A[h * C:(h + 1) * C, H + h] = np.asarray(inputs[f"a_dst{l}"], np.float32)[h]
        f[f"Waug{l}"] = np.concatenate([W, W @ A], axis=1).astype(np.float16)
        f[f"brep{l}"] = np.tile(np.asarray(inputs[f"b{l}"], np.float16)[None, :],
                                (128, 1))
    f["fc1_w"] = np.asarray(inputs["fc1_w"], np.float32)
    f["fc1_b"] = np.asarray(inputs["fc1_b"], np.float32).reshape(-1, 1)
    f["out_w"] = np.asarray(inputs["out_w"], np.float32)
    f["out_b"] = np.asarray(inputs["out_b"], np.float32).reshape(-1, 1)
    return f


def build_program(cfg):
    from concourse import bacc, bass, mybir, tile
    from concourse.masks import make_identity
    f32, f16, i16 = mybir.dt.float32, mybir.dt.float16, mybir.dt.int16
    AF, ALU = mybir.ActivationFunctionType, mybir.AluOpType
    G, NC, FEAT = cfg["G"], cfg["NC"], cfg["FEAT"]
    NLOC, NSL, NTL, GSL = cfg["NLOC"], cfg["NSL"], cfg["NTL"], cfg["GSL"]
    T0, T1, HALF0, HALF1 = cfg["T0"], cfg["T1"], cfg["HALF0"], cfg["HALF1"]
    TOT, PAD3 = cfg["TOT"], cfg["PAD3"]
    NW, NT3, WIN = cfg["NW"], cfg["NT3"], cfg["WIN"]
    NPW = 128 // WIN
    bsz, offs, groups, GC = cfg["bsz"], cfg["offs"], cfg["groups"], cfg["GC"]
    lo_shared, rmap = cfg["lo_shared"], cfg["rmap"]
    core_ids = list(range(NC))

    nc = bacc.Bacc(None, num_devices=NC, num_swdge_queues=2)

    featT = nc.declare_dram_parameter("featT", [FEAT, GSL], f16, False)
    featTo = nc.declare_dram_parameter("featTown", [FEAT, NSL], f16, False)
    Waug, brep = [], []
    for l in (1, 2, 3):
        Waug.append(nc.declare_dram_parameter(
            f"Waug{l}", [FEAT if l == 1 else HC, HC + 2 * H], f16, False))
        brep.append(nc.declare_dram_parameter(f"brep{l}", [128, HC], f16, False))
    fc1_w = nc.declare_dram_parameter("fc1_w", [2 * HC, 48], f32, False)
    fc1_b = nc.declare_dram_parameter("fc1_b", [48, 1], f32, False)
    out_w = nc.declare_dram_parameter("out_w", [48, 2], f32, False)
    out_b = nc.declare_dram_parameter("out_b", [2, 1], f32, False)
    inv_cnt = nc.declare_dram_parameter("inv_cnt", [96, G], f32, False)
    g1i = nc.declare_dram_parameter("g1", [128, TOT // 16], i16, False)
    g2i = nc.declare_dram_parameter("g2", [128, TOT // 16], i16, False)
    dstvi = nc.declare_dram_parameter("dstv", [128, TOT // 128], f16, False)
    s3i = nc.declare_dram_parameter("s3", [128, NSL // 16], i16, False)
    cmbi = nc.declare_dram_parameter("cmb", [128, 6], f32, False)
    wplacei = nc.declare_dram_parameter("wplace", [128, 8], i16, False)
    onehoti = nc.declare_dram_parameter("onehot", [NTL, 128, G], f16, False)
    yout = nc.declare_dram_parameter("y", [2, G], f32, True)

    htab = [nc.dram_tensor("htab0", [NC, HALF0, ROW], f16, addr_space="Shared"),
            nc.dram_tensor("htab1", [NC, HALF1, ROW], f16, addr_space="Shared")]
    own = [nc.dram_tensor("own0", [HALF0, ROW], f16),
           nc.dram_tensor("own1", [HALF1, ROW], f16)]
    stab = nc.dram_tensor("stab", [NSL, SROW], f16)
    padgrid = nc.dram_tensor("padgrid", [PAD3 + 128, HC], f32)
    maxgrid = nc.dram_tensor("maxgrid", [G + 1, HC], f32)
    poolsl = nc.dram_tensor("poolsl", [96, 4, G], f32)
    poolag = nc.dram_tensor("poolag", [NC, 96, 4, G], f32, addr_space="Shared")

    htabv = [htab[0][:].rearrange("c n e -> (c n) e"),
             htab[1][:].rearrange("c n e -> (c n) e")]

    with tile.TileContext(nc) as tc:
        with (
            tc.tile_pool(name="const", bufs=1) as constp,
            tc.tile_pool(name="wpool", bufs=1) as wpool,
            tc.tile_pool(name="lhs", bufs=3) as lhsp,
            tc.tile_pool(name="dense", bufs=3) as densep,
            tc.tile_pool(name="edge", bufs=3) as edgep,
            tc.tile_pool(name="edgec", bufs=2) as edgecp,
            tc.tile_pool(name="accp", bufs=1) as accp,
            tc.tile_pool(name="stagp", bufs=1) as stagp,
            tc.tile_pool(name="post", bufs=2) as postp,
            tc.tile_pool(name="xt", bufs=3) as xtp,
            tc.tile_pool(name="psum", bufs=2, space="PSUM") as psump,
            tc.tile_pool(name="psumP", bufs=1, space="PSUM") as psumPp,
            tc.tile_pool(name="small", bufs=1) as smallp,
        ):
            ident16 = constp.tile([128, 128], f16)
            make_identity(nc, ident16[:])
            identf = constp.tile([128, 128], f32)
            make_identity(nc, identf[:])
            iotai = constp.tile([128, 128], mybir.dt.int32)
            nc.gpsimd.iota(iotai[:], pattern=[[1, 128]], base=0,
                           channel_multiplier=0)
            iota16 = constp.tile([128, 128], f16)
            nc.vector.tensor_copy(iota16[:], iotai[:])
            eshift = constp.tile([128, 1], f32)
            nc.vector.memset(eshift[:], ESHIFT)
            zbias = constp.tile([128, 1], f32)
            nc.vector.memset(zbias[:], 0.0)

            wtA, wtB, bt = [], [], []
            for l in range(3):
                ka = FEAT if l == 0 else 96
                a = wpool.tile([ka, HC + 2 * H], f16, tag=f"wtA{l}")
                nc.sync.dma_start(a[:], Waug[l][:ka])
                wtA.append(a)
                if l == 0:
                    wtB.append(None)
                else:
                    b_ = wpool.tile([96, HC + 2 * H], f16, tag=f"wtB{l}")
                    nc.sync.dma_start(b_[:], Waug[l][96:])
                    wtB.append(b_)
                bb = wpool.tile([128, HC], f16, tag=f"bt{l}")
                nc.sync.dma_start(bb[:], brep[l][:])
                bt.append(bb)
            idxt = {}
            for nm, dram, w_ in (("g1", g1i, TOT // 16), ("g2", g2i, TOT // 16),
                                 ("s3", s3i, NSL // 16)):
                t = wpool.tile([128, w_], i16, tag=f"ix{nm}")
                nc.sync.dma_start(t[:], dram[:])
                idxt[nm] = t
            dstvt = wpool.tile([128, TOT // 128], f16, tag="dstv")
            nc.sync.dma_start(dstvt[:], dstvi[:])
            cmbt = wpool.tile([128, 6], f32, tag="cmb")
            nc.sync.dma_start(cmbt[:], cmbi[:])
            wplt = wpool.tile([128, 8], i16, tag="wpl")
            nc.sync.dma_start(wplt[:], wplacei[:])
            invt = wpool.tile([96, G], f32, tag="inv")
            nc.sync.dma_start(invt[:], inv_cnt[:])
            fc1wt = []
            for k in range(4):
                t = wpool.tile([96, 48], f32, tag=f"fc1{k}")
                nc.sync.dma_start(t[:], fc1_w[k * 96:(k + 1) * 96])
                fc1wt.append(t)
            fc1bt = wpool.tile([48, 1], f32, tag="fc1b")
            nc.sync.dma_start(fc1bt[:], fc1_b[:])
            outwt = wpool.tile([48, 2], f32, tag="outw")
            nc.sync.dma_start(outwt[:], out_w[:])
            outbt = wpool.tile([2, 1], f32, tag="outb")
            nc.sync.dma_start(outbt[:], out_b[:])

            acc = accp.tile([128, NTL, HC + H], f16, tag="acc")
            sdc = accp.tile([128, TOT // 128, H], f16, tag="sdc")
            stg_state = {"tile": None, "base": 0}
            stabst = stagp.tile([128, NTL, H], f16, tag="stabst")

            zt = constp.tile([128, 4, HC], f32, tag="zt")
            nc.vector.memset(zt[:], 0.0)
            r0 = 0
            while r0 < PAD3 + 128:
                rr = min(512, PAD3 + 128 - r0)
                nc.sync.dma_start(
                    padgrid[r0:r0 + rr].rearrange("(p a) c -> p (a c)", p=128),
                    zt[:, :rr // 128].rearrange("p a c -> p (a c)"))
                r0 += rr

            # ---- layer-1 stab (own s_dst rows from the per-core features)
            for t in range(NTL):
                lhs = lhsp.tile([FEAT, 128], f16, tag="slhs")
                nc.sync.dma_start(lhs[:], featTo[:, t * 128:(t + 1) * 128])
                ps = psump.tile([128, H], f32, tag="dps")
                nc.tensor.matmul(ps[:], lhs[:], wtA[0][:, HC + H:],
                                 start=True, stop=True)
                nc.scalar.activation(stabst[:, t], ps[:], AF.Copy)
            nc.sync.dma_start(
                stab[:].rearrange("(t p) c -> p t c", p=128)[:, :, :H],
                stabst[:])

            # ---- layer-1 dense-full: every core computes the whole htab
            BT = 4
            for half in (0, 1):
                ntile = NC * (T0 if half == 0 else T1)
                hv = htabv[half]
                for b0 in range(0, ntile, BT):
                    nb = min(BT, ntile - b0)
                    lhs = lhsp.tile([FEAT, BT * 128], f16, tag="dlhs")
                    nc.sync.dma_start(
                        lhs[:, :nb * 128],
                        featT[:, (half * NC * HALF0) + b0 * 128:
                              (half * NC * HALF0) + (b0 + nb) * 128])
                    hrow = densep.tile([128, BT, ROW], f16, tag="hrow")
                    for j in range(nb):
                        ps = psump.tile([128, HC + 2 * H], f32, tag="dps")
                        nc.tensor.matmul(ps[:], lhs[:, j * 128:(j + 1) * 128],
                                         wtA[0][:], start=True, stop=True)
                        nc.vector.tensor_copy(hrow[:, j, :HC + 2 * H], ps[:])
                    nc.scalar.dma_start(
                        hv[b0 * 128:(b0 + nb) * 128]
                        .rearrange("(j p) e -> p j e", p=128),
                        hrow[:, :nb])

            def post_tile(l, t, tmp):
                """tmp: SBUF fp32 [128, HC+H] aggregated messages + denoms."""
                den = postp.tile([128, H], f32, tag="pden")
                nc.vector.tensor_scalar(den[:], tmp[:, HC:], 1e-16, None,
                                        ALU.max)
                nc.vector.reciprocal(den[:], den[:])
                y16 = postp.tile([128, HC], f16, tag="py")
                for h in range(H):
                    nc.vector.tensor_scalar(
                        y16[:, h * C:(h + 1) * C], tmp[:, h * C:(h + 1) * C],
                        den[:, h:h + 1], None, ALU.mult)
                nc.vector.scalar_tensor_tensor(
                    y16[:], y16[:], 1.0, bt[l][:], ALU.bypass, ALU.add)
                e16 = postp.tile([128, HC], f16, tag="pe")
                nc.vector.tensor_scalar(e16[:], y16[:], 0.0, None, ALU.min)
                nc.scalar.activation(e16[:], e16[:], AF.Exp)
                nc.vector.scalar_tensor_tensor(
                    y16[:], e16[:], -1.0, y16[:], ALU.add, ALU.max)
                return y16

            def own_rows(l, t, y16):
                """Transpose y16 and compute next layer's own htab/stab rows."""
                xt = xtp.tile([96, 2, 128], f16, tag="xt")
                for blk in range(2):
                    pt = psump.tile([96, 128], f16, tag="tps")
                    nc.tensor.transpose(
                        pt[:], y16[:, blk * 96:(blk + 1) * 96], ident16[:])
                    nc.scalar.activation(xt[:, blk], pt[:], AF.Copy)
                ps2 = psump.tile([128, HC + 2 * H], f32, tag="dps")
                nc.tensor.matmul(ps2[:], xt[:, 0], wtA[l + 1][:],
                                 start=True, stop=False)
                nc.tensor.matmul(ps2[:], xt[:, 1], wtB[l + 1][:],
                                 start=False, stop=True)
                half = 0 if t < T0 else 1
                tl = t if t < T0 else t - T0
                if stg_state["tile"] is None:
                    stg_state["tile"] = stagp.tile([128, 4, ROW], f16,
                                                   tag="stg", name="stg")
                    stg_state["base"] = tl
                stg = stg_state["tile"]
                nc.scalar.activation(
                    stg[:, tl - stg_state["base"], :HC + 2 * H], ps2[:],
                    AF.Copy)
                nc.scalar.activation(stabst[:, t], ps2[:, HC + H:], AF.Copy)
                last_of_half = t == T0 - 1 or t == NTL - 1
                if tl - stg_state["base"] == 3 or last_of_half:
                    nch = tl - stg_state["base"] + 1
                    nc.sync.dma_start(
                        own[half][stg_state["base"] * 128:
                                  (stg_state["base"] + nch) * 128]
                        .rearrange("(t p) e -> p t e", p=128),
                        stg[:, :nch])
                    stg_state["tile"] = None
                if t == T0 - 1:
                    nc.gpsimd.collective_compute(
                        "AllGather", ALU.bypass, replica_groups=[core_ids],
                        ins=[own[0][:]], outs=[htab[0][:]])
                if t == NTL - 1:
                    nc.sync.dma_start(
                        stab[:].rearrange("(t p) c -> p t c", p=128)[:, :, :H],
                        stabst[:])

            wmax = stagp.tile([96, 2, NW], f32, tag="wmax")

            def wmax_tile(r):
                rows = postp.tile([128, HC], f32, tag="pyo")
                nc.sync.dma_start(rows[:], padgrid[r * 128:(r + 1) * 128])
                for blk in range(2):
                    pt = psump.tile([96, 128], f32, tag="tps")
                    nc.tensor.transpose(
                        pt[:], rows[:, blk * 96:(blk + 1) * 96], identf[:])
                    nc.vector.tensor_reduce(
                        wmax[:, blk, r * NPW:(r + 1) * NPW],
                        pt[:].rearrange("p (w q) -> p w q", q=WIN),
                        mybir.AxisListType.X, ALU.max)

            ohall = wpool.tile([128, NTL, G], f16, tag="ohall")
            nc.sync.dma_start(
                ohall[:], onehoti[:].rearrange("t p g -> p t g"))

            def pool_tile(t, y16, sump):
                for blk in range(2):
                    nc.tensor.matmul(
                        sump[blk][:], y16[:, blk * 96:(blk + 1) * 96],
                        ohall[:, t], start=(t == 0), stop=(t == NTL - 1))
                yo = postp.tile([128, HC], f32, tag="pyo")
                nc.vector.tensor_scalar(yo[:], y16[:], MAXOFF, None, ALU.add)
                nc.gpsimd.dma_scatter_add(
                    padgrid[int(lo_shared[t]):, :],
                    yo[:].rearrange("p (a c) -> p a c", a=1),
                    idxt["s3"][:, t * 8:(t + 1) * 8], 128, 128, HC,
                    single_packet=False)

            def edge_sd_group(h, tt, i0, S):
                cols = S // 128
                sd = edgep.tile([128, GC, SROW], f16, tag="sd")
                sbase = tt[0] * 128
                nc.gpsimd.dma_gather(
                    sd[:, :cols], stab[sbase:sbase + len(tt) * 128],
                    idxt["g2"][:, i0 // 16:(i0 + S) // 16], S, S, SROW,
                    single_packet=False, queue_num=1)
                nc.vector.tensor_copy(
                    sdc[:, i0 // 128:i0 // 128 + cols], sd[:, :cols, :H])

            def edge_sd_phase(l, half):
                for (h, tt, i0, S) in groups:
                    if h == half:
                        edge_sd_group(h, tt, i0, S)

            def edge_pass(l, half, sump, inline_sd=False):
                for (h, tt, i0, S) in groups:
                        if h != half:
                            continue
                        if inline_sd:
                            edge_sd_group(h, tt, i0, S)
                        cols = S // 128
                        big = edgep.tile([128, GC, ROW], f16, tag="big")
                        nc.gpsimd.dma_gather(
                            big[:, :cols], htabv[half],
                            idxt["g1"][:, i0 // 16:(i0 + S) // 16], S, S, ROW,
                            single_packet=False, queue_num=0)
                        u = edgecp.tile([128, GC, H], f32, tag="u")
                        nc.vector.scalar_tensor_tensor(
                            u[:, :cols], big[:, :cols, HC:HC + H], 1.0,
                            sdc[:, i0 // 128:i0 // 128 + cols],
                            ALU.bypass, ALU.add)
                        nc.vector.scalar_tensor_tensor(
                            u[:, :cols], u[:, :cols], NEG, u[:, :cols],
                            ALU.mult, ALU.max)
                        w4 = edgecp.tile([128, GC, H], f16, tag="w4")
                        nc.scalar.activation(w4[:, :cols], u[:, :cols],
                                             AF.Exp, bias=eshift[:])
                        WSUB = 8
                        for c0 in range(0, cols, WSUB):
                            cc = min(WSUB, cols - c0)
                            wfull = edgecp.tile([128, WSUB, HC], f16,
                                               tag="wfull")
                            nc.scalar.activation(
                                wfull[:, :cc].rearrange(
                                    "p c (h k) -> p c h k", h=H),
                                w4[:, c0:c0 + cc, :, None]
                                .to_broadcast([128, cc, H, C]), AF.Copy)
                            nc.vector.tensor_tensor(
                                big[:, c0:c0 + cc, :HC],
                                big[:, c0:c0 + cc, :HC],
                                wfull[:, :cc], ALU.mult)
                        nc.vector.tensor_copy(big[:, :cols, HC:HC + H],
                                              w4[:, :cols])
                        oh = edgecp.tile([128, GC, 128], f16, tag="oh")
                        nc.vector.tensor_tensor(
                            oh[:, :cols],
                            iota16[:, None, :].to_broadcast([128, cols, 128]),
                            dstvt[:, i0 // 128:i0 // 128 + cols, None]
                            .to_broadcast([128, cols, 128]),
                            ALU.is_equal)
                        q0 = 0
                        for t in tt:
                            ncq = int(bsz[half, t]) // 128
                            ps = psump.tile([128, HC + H], f32, tag="agg")
                            for q in range(ncq):
                                nc.tensor.matmul(
                                    ps[:], oh[:, q0 + q],
                                    big[:, q0 + q, :HC + H],
                                    start=(q == 0), stop=(q == ncq - 1))
                            q0 += ncq
                            if half == 0:
                                nc.scalar.activation(acc[:, t], ps[:], AF.Copy)
                            else:
                                tmp = postp.tile([128, HC + H], f32, tag="tmp")
                                nc.vector.scalar_tensor_tensor(
                                    tmp[:], ps[:], 1.0, acc[:, t],
                                    ALU.bypass, ALU.add)
                                y16 = post_tile(l, t, tmp)
                                if l < 2:
                                    own_rows(l, t, y16)
                                else:
                                    pool_tile(t, y16, sump)

            sump0 = psumPp.tile([96, G], f32, tag="sum0")
            sump1 = psumPp.tile([96, G], f32, tag="sum1")
            sump = [sump0, sump1]
            edge_pass(0, 0, None, inline_sd=True)
            for l in range(3):
                edge_pass(l, 1, sump if l == 2 else None,
                          inline_sd=(l == 0))
                if l < 2:
                    edge_sd_phase(l + 1, 0)
                    edge_pass(l + 1, 0, None)
                    edge_sd_phase(l + 1, 1)
                    nc.gpsimd.collective_compute(
                        "AllGather", ALU.bypass, replica_groups=[core_ids],
                        ins=[own[1][:]], outs=[htab[1][:]])

            # ---- pooling epilogue (fp32, as v1)
            for r in range(NT3):
                wmax_tile(r)
            wrow = smallp.tile([128, HC], f32, tag="wrow")
            for blk in range(2):
                pt2 = psump.tile([128, 96], f32, tag="tps")
                nc.tensor.transpose(pt2[:NW], wmax[:, blk],
                                    identf[:96, :96])
                nc.vector.tensor_copy(
                    wrow[:NW, blk * 96:(blk + 1) * 96], pt2[:NW])
            for ki, s in enumerate([1, 2, 4, 8, 16, 32]):
                if s >= NW:
                    break
                sh = smallp.tile([128, HC], f32, tag="wsh")
                nc.sync.dma_start(sh[:NW - s], wrow[s:NW])
                nc.vector.tensor_scalar(sh[:NW - s], sh[:NW - s],
                                        cmbt[:NW - s, ki:ki + 1],
                                        None, ALU.add)
                nc.vector.tensor_max(wrow[:NW - s], wrow[:NW - s],
                                     sh[:NW - s])
            zg = smallp.tile([G + 1, HC], f32, tag="zg")
            nc.vector.memset(zg[:], 0.0)
            nc.sync.dma_start(maxgrid[:], zg[:])
            nc.gpsimd.dma_scatter_add(
                maxgrid[:], wrow[:].rearrange("p (a c) -> p a c", a=1),
                wplt[:], 128, 128, HC, single_packet=False)
            mg = smallp.tile([G, HC], f32, tag="mg")
            nc.sync.dma_start(mg[:], maxgrid[:G])
            pp = smallp.tile([96, 4, G], f32, tag="pp")
            for blk in range(2):
                nc.vector.tensor_copy(pp[:, blk], sump0[:] if blk == 0
                                      else sump1[:])
                pt3 = psump.tile([96, G], f32, tag="tps")
                nc.tensor.transpose(
                    pt3[:], mg[:, blk * 96:(blk + 1) * 96],
                    identf[:G, :G])
                nc.vector.tensor_copy(pp[:, 2 + blk], pt3[:])
            nc.sync.dma_start(poolsl[:], pp[:])
            nc.gpsimd.collective_compute(
                "AllGather", ALU.bypass,
                replica_groups=[core_ids],
                ins=[poolsl[:]], outs=[poolag[:]])
            agg2 = smallp.tile([96, 4, G], f32, tag="agg2")
            for c_ in range(NC):
                at = smallp.tile([96, 4, G], f32, tag="agt")
                nc.sync.dma_start(at[:], poolag[c_])
                if c_ == 0:
                    nc.vector.tensor_copy(agg2[:], at[:])
                else:
                    nc.vector.tensor_add(agg2[:, :2], agg2[:, :2],
                                         at[:, :2])
                    nc.vector.tensor_max(agg2[:, 2:], agg2[:, 2:],
                                         at[:, 2:])
            for blk in range(2):
                nc.vector.tensor_mul(agg2[:, blk], agg2[:, blk], invt[:])
                nc.vector.tensor_scalar(agg2[:, 2 + blk],
                                        agg2[:, 2 + blk],
                                        -MAXOFF, None, ALU.add)
            zp = psump.tile([48, G], f32, tag="tps")
            for k in range(4):
                nc.tensor.matmul(zp[:], fc1wt[k], agg2[:, k],
                                 start=(k == 0), stop=(k == 3))
            z = smallp.tile([48, G], f32, tag="z")
            nc.vector.tensor_scalar(z[:], zp[:], fc1bt[:], None,
                                    ALU.add)
            e2 = smallp.tile([48, G], f32, tag="e2")
            nc.vector.tensor_scalar(e2[:], z[:], 0.0, None, ALU.min)
            nc.scalar.activation(e2[:], e2[:], AF.Exp)
            nc.vector.tensor_scalar(e2[:], e2[:], -1.0, None, ALU.add)
            nc.vector.tensor_max(z[:], z[:], e2[:])
            yp = psump.tile([2, G], f32, tag="tps")
            nc.tensor.matmul(yp[:], outwt[:], z[:], start=True,
                             stop=True)
            yf = smallp.tile([2, G], f32, tag="yf")
            nc.vector.tensor_scalar(yf[:], yp[:], outbt[:], None,
                                    ALU.add)
            nc.sync.dma_start(yout[:], yf[:])
    nc.finalize()
    return nc


def run(inputs, cfg, **run_kw):
    data, inv_cnt = host_prep(cfg, inputs["adj"], inputs["batch"])
    fl = prep_float_inputs(cfg, inputs)
    NC, NLOC, NSL = cfg["NC"], cfg["NLOC"], cfg["NSL"]
    in_maps = []
    for c in range(NC):
        m = dict(fl)
        m["inv_cnt"] = inv_cnt
        fto = np.zeros((cfg["FEAT"], NSL), np.float16)
        fto[:, :NLOC] = np.asarray(inputs["features"],
                                   np.float32)[c * NLOC:(c + 1) * NLOC].T
        m["featTown"] = fto
        m.update({k: data[c][k] for k in
                  ("g1", "g2", "dstv", "s3", "cmb", "wplace", "onehot")})
        in_maps.append(m)
    nc = build_program(cfg)
    from concourse.bass_utils import run_bass_kernel_spmd
    res = run_bass_kernel_spmd(nc, in_maps, list(range(NC)), **run_kw)
    y = np.asarray(res.results[0]["y"])
    return y.T.copy(), res


def kernel(**inputs):
    y, _ = run(inputs, make_cfg())
    return y


# revision 4
# speedup vs baseline: 1.0473x; 1.0264x over previous
"""3-layer GAT + graph pooling + MLP on 8 Trainium2 NeuronCores (Bass), v2.

Sharding: core c owns dst-nodes [c*NLOC, (c+1)*NLOC) and their in-edges.
All node tables are fp16; accumulation stays fp32 in PSUM.

Per layer:
  htab:   [h(192) | s_src(4) | s_dst(4) | pad] fp16 rows, 512B each, split in
          two half-tables (<=32768 rows each, int16 gather indices).  Layer 1
          computes it locally from the replicated features; layers 2-3
          AllGather per-core-computed own rows (h = x@Waug fused with the
          attention s columns), chunked in halves so the collective overlaps
          the previous layer's edge phase.
  edge:   two passes (one per half-table).  Per group of dst tiles:
          dma_gather 512B h-rows + 256B s_dst rows, w = exp(leaky(s)+shift)
          on ACT (fp16), scale h rows in place, one-hot via a single blocked
          is_equal, aggregate per 128-edge column on the TensorEngine:
          psum += onehot(dst)^T @ [w*h | w].  Pass 0 spills psum to an SBUF
          accumulator; pass 1 adds it back and runs post (softmax divide,
          bias, ELU) + transpose + the NEXT layer's own-row matmuls.
Pooling: as in v1 (unique-index scatter into graph-aligned slots, one-hot
sum pool, windowed max pool, small AllGather, replicated MLP).
All data-dependent structure (indices, one-hots) is INPUT DATA; the program
is static and identical across cores (SPMD).
"""

import sys
import numpy as np

sys.path.insert(0, "/opt/trn_rl_repo")

H, C = 4, 48
HC = H * C          # 192
NEG = 0.2
ESHIFT = -6.0       # global softmax shift: exp(leaky(u)+ESHIFT) fits fp16
BIGNEG = -2.0e30
MAXOFF = 1000.0     # max-pool offset: x3 = elu(...) >= -1, so x3+1000 > 0
ROW = 256           # htab row: h 192 | s_src 4 | s_dst 4 | pad (fp16)
SROW = 128          # stab row (fp16): s_dst 4 | pad  -> 256B
GROUP = 3           # dst tiles per gather call


def make_cfg(N=50000, E=800000, G=64, NC=8, FEAT=128, WIN=64):
    NLOC = N // NC
    assert NLOC * NC == N
    NTL = (NLOC + 127) // 128
    NSL = NTL * 128
    T0 = min((NTL + 1) // 2, max(1, int(NTL * 0.35)))
    T1 = NTL - T0
    assert T1 >= 1
    HALF0, HALF1 = T0 * 128, T1 * 128
    assert NC * HALF0 <= 32768 and NC * HALF1 <= 32768
    return dict(N=N, E=E, G=G, NC=NC, FEAT=FEAT, NLOC=NLOC, NSL=NSL, NTL=NTL,
                T0=T0, T1=T1, HALF0=HALF0, HALF1=HALF1, GSL=NC * NSL, WIN=WIN)


def _wrap_idx(idx):
    """SWDGE idx layout: element i -> [i % 16, i // 16], replicated to 128
    partitions (one copy per Q7 core)."""
    T = idx.shape[0]
    out = np.ascontiguousarray(idx.reshape(T // 16, 16).T).astype(np.int16)
    return np.tile(out, (8, 1))


def host_prep(cfg, adj, batch):
    N, G, NC = cfg["N"], cfg["G"], cfg["NC"]
    NLOC, NTL, WIN = cfg["NLOC"], cfg["NTL"], cfg["WIN"]
    NSL, T0, HALF0, HALF1 = cfg["NSL"], cfg["T0"], cfg["HALF0"], cfg["HALF1"]
    src = np.asarray(adj[0], dtype=np.int64)
    dst = np.asarray(adj[1], dtype=np.int64)
    batch = np.asarray(batch, dtype=np.int64)
    c_src = src // NLOC
    n_src = src % NLOC
    half_src = (n_src >= HALF0).astype(np.int64)
    row_src = np.where(half_src == 0, c_src * HALF0 + n_src,
                       c_src * HALF1 + (n_src - HALF0))
    counts_g = np.bincount(batch, minlength=G)

    # ---- edge grouping: (half, dst-tile) blocks, each padded %128, >=128
    blocks_all = []
    sizes = np.zeros((NC, 2, NTL), dtype=np.int64)
    for c in range(NC):
        lo = c * NLOC
        esel = np.nonzero((dst >= lo) & (dst < lo + NLOC))[0]
        dt_of = (dst[esel] - lo) // 128
        hf_of = half_src[esel]
        d = {}
        for h in range(2):
            for t in range(NTL):
                ee = esel[(hf_of == h) & (dt_of == t)]
                d[(h, t)] = ee
                sizes[c, h, t] = len(ee)
        blocks_all.append(d)
    bsz = np.zeros((2, NTL), dtype=np.int64)
    for h in range(2):
        for t in range(NTL):
            m = max(int(sizes[:, h, t].max()), 1)
            bsz[h, t] = -(-m // 128) * 128
    offs = np.zeros((2, NTL), dtype=np.int64)
    o = 0
    for h in range(2):
        for t in range(NTL):
            offs[h, t] = o
            o += bsz[h, t]
    TOT = int(o)

    # gather groups: (half, [tiles], i0, S) — every dst tile has a block in
    # each src half-table; greedy-pack tiles up to a column cap
    CAP = 30 * 128
    groups = []
    for h in range(2):
        tt, S = [], 0
        for t in range(NTL):
            b = int(bsz[h, t])
            if tt and S + b > CAP:
                groups.append((h, list(tt), int(offs[h, tt[0]]), S))
                tt, S = [], 0
            tt.append(t)
            S += b
        if tt:
            groups.append((h, list(tt), int(offs[h, tt[0]]), S))
    GC = max(S // 128 for (_, _, _, S) in groups)

    # ---- layer-3 graph-aligned slots (as v1)
    pad3_meta, pad3_tot = [], 0
    for c in range(NC):
        lo = c * NLOC
        b = batch[lo:lo + NLOC]
        gids, starts = np.unique(b, return_index=True)
        osort = np.argsort(starts)
        gids, starts = gids[osort], starts[osort]
        ends = np.append(starts[1:], NLOC)
        slots = np.empty(NLOC, dtype=np.int64)
        wg, fwin = [], []
        pos = 0
        for g, s, e in zip(gids, starts, ends):
            cnt = e - s
            slots[s:e] = pos + np.arange(cnt)
            nw = -(-cnt // WIN)
            wg += [int(g)] * nw
            fwin += [1] + [0] * (nw - 1)
            pos += nw * WIN
        pad3_meta.append((slots, wg, fwin))
        pad3_tot = max(pad3_tot, pos)
    PAD3 = -(-pad3_tot // 128) * 128
    NW, NT3 = PAD3 // WIN, PAD3 // 128
    assert NW <= 128
    # shared (min over cores) scatter lower bound per dst tile, for sliced
    # scatter APs so pooling window reductions can start before all tiles land
    lo_shared = np.full(NTL, 10**9, dtype=np.int64)
    for c in range(NC):
        slots3 = pad3_meta[c][0]
        for t in range(NTL):
            sl = slots3[t * 128:min((t + 1) * 128, NLOC)]
            if len(sl):
                lo_shared[t] = min(lo_shared[t], int(sl.min()))
    lo_shared = np.minimum.accumulate(lo_shared[::-1])[::-1]
    lo_shared[0] = 0
    rmap = np.zeros(NT3, dtype=np.int64)
    for r in range(NT3):
        sel = np.nonzero(lo_shared <= (r + 1) * 128 - 1)[0]
        rmap[r] = int(sel.max()) if len(sel) else 0
    cfg.update(TOT=TOT, bsz=bsz, offs=offs, groups=groups, GC=GC,
               PAD3=PAD3, NW=NW, NT3=NT3, lo_shared=lo_shared, rmap=rmap)

    gbase = {}
    for (h, tt, i0, S) in groups:
        for t in tt:
            gbase[(h, t)] = tt[0]

    data = []
    strides = [1, 2, 4, 8, 16, 32]
    for c in range(NC):
        lo = c * NLOC
        slots3, wg, fwin = pad3_meta[c]
        g1 = np.zeros(TOT, dtype=np.int64)
        g2 = np.zeros(TOT, dtype=np.int64)
        dstv = np.full(TOT, 999.0, dtype=np.float16)
        for h in range(2):
            for t in range(NTL):
                ee = blocks_all[c][(h, t)]
                i0 = int(offs[h, t])
                g1[i0:i0 + len(ee)] = row_src[ee]
                g2[i0:i0 + len(ee)] = (dst[ee] - lo) - gbase[(h, t)] * 128
                dstv[i0:i0 + len(ee)] = ((dst[ee] - lo) % 128).astype(np.float16)
        assert g1.min() >= 0 and g2.min() >= 0
        s3 = np.full(NSL, PAD3, dtype=np.int64)
        s3[:NLOC] = slots3
        for t in range(NTL):
            s3[t * 128:(t + 1) * 128] -= lo_shared[t]
        assert s3.min() >= 0
        wgp = np.full(NW, -1, dtype=np.int64)
        wgp[:len(wg)] = wg
        cmb = np.full((128, len(strides)), BIGNEG, dtype=np.float32)
        for k, s in enumerate(strides):
            for i in range(NW - s):
                if wgp[i] >= 0 and wgp[i] == wgp[i + s]:
                    cmb[i, k] = 0.0
        wplace = np.full(128, G, dtype=np.int64)
        for i in range(len(wg)):
            if fwin[i]:
                wplace[i] = wg[i]
        onehot = np.zeros((NTL, 128, G), dtype=np.float16)
        nn = np.arange(NLOC)
        onehot[nn // 128, nn % 128, batch[lo:lo + NLOC]] = 1.0
        data.append(dict(
            g1=_wrap_idx(g1),
            g2=_wrap_idx(g2),
            dstv=np.ascontiguousarray(
                dstv.reshape(TOT // 128, 128).T).astype(np.float16),
            s3=_wrap_idx(s3),
            cmb=cmb,
            wplace=_wrap_idx(wplace),
            onehot=onehot,
        ))
    inv_cnt = np.tile((1.0 / np.maximum(counts_g, 1.0))
                      .astype(np.float32)[None, :], (96, 1))
    return data, inv_cnt


def prep_float_inputs(cfg, inputs):
    NC, NLOC, NSL, FEAT = cfg["NC"], cfg["NLOC"], cfg["NSL"], cfg["FEAT"]
    HALF0, HALF1, GSL = cfg["HALF0"], cfg["HALF1"], cfg["GSL"]
    f = {}
    feat = np.asarray(inputs["features"], np.float32)
    fpad = np.zeros((NC, NSL, FEAT), np.float32)
    for c in range(NC):
        fpad[c, :NLOC] = feat[c * NLOC:(c + 1) * NLOC]
    h0 = fpad[:, :HALF0].reshape(NC * HALF0, FEAT)
    h1 = fpad[:, HALF0:].reshape(NC * HALF1, FEAT)
    f["featT"] = np.ascontiguousarray(
        np.concatenate([h0, h1], axis=0).T).astype(np.float16)
    for l in (1, 2, 3):
        W = np.asarray(inputs[f"W{l}"], np.float32)
        A = np.zeros((HC, 2 * H), np.float32)
        for h in range(H):
            A[h * C:(h + 1) * C, h] = np.asarray(inputs[f"a_src{l}"], np.float32)[h]
            A[h * C:(h + 1) * C, H + h] = np.asarray(inputs[f"a_dst{l}"], np.float32)[h]
        f[f"Waug{l}"] = np.concatenate([W, W @ A], axis=1).astype(np.float16)
        f[f"brep{l}"] = np.tile(np.asarray(inputs[f"b{l}"], np.float16)[None, :],
                                (128, 1))
    f["fc1_w"] = np.asarray(inputs["fc1_w"], np.float32)
    f["fc1_b"] = np.asarray(inputs["fc1_b"], np.float32).reshape(-1, 1)
    f["out_w"] = np.asarray(inputs["out_w"], np.float32)
    f["out_b"] = np.asarray(inputs["out_b"], np.float32).reshape(-1, 1)
    return f


def build_program(cfg):
    from concourse import bacc, bass, mybir, tile
    from concourse.masks import make_identity
    f32, f16, i16 = mybir.dt.float32, mybir.dt.float16, mybir.dt.int16
    AF, ALU = mybir.ActivationFunctionType, mybir.AluOpType
    G, NC, FEAT = cfg["G"], cfg["NC"], cfg["FEAT"]
    NLOC, NSL, NTL, GSL = cfg["NLOC"], cfg["NSL"], cfg["NTL"], cfg["GSL"]
    T0, T1, HALF0, HALF1 = cfg["T0"], cfg["T1"], cfg["HALF0"], cfg["HALF1"]
    TOT, PAD3 = cfg["TOT"], cfg["PAD3"]
    NW, NT3, WIN = cfg["NW"], cfg["NT3"], cfg["WIN"]
    NPW = 128 // WIN
    bsz, offs, groups, GC = cfg["bsz"], cfg["offs"], cfg["groups"], cfg["GC"]
    lo_shared, rmap = cfg["lo_shared"], cfg["rmap"]
    core_ids = list(range(NC))

    nc = bacc.Bacc(None, num_devices=NC, num_swdge_queues=2)

    featT = nc.declare_dram_parameter("featT", [FEAT, GSL], f16, False)
    featTo = nc.declare_dram_parameter("featTown", [FEAT, NSL], f16, False)
    Waug, brep = [], []
    for l in (1, 2, 3):
        Waug.append(nc.declare_dram_parameter(
            f"Waug{l}", [FEAT if l == 1 else HC, HC + 2 * H], f16, False))
        brep.append(nc.declare_dram_parameter(f"brep{l}", [128, HC], f16, False))
    fc1_w = nc.declare_dram_parameter("fc1_w", [2 * HC, 48], f32, False)
    fc1_b = nc.declare_dram_parameter("fc1_b", [48, 1], f32, False)
    out_w = nc.declare_dram_parameter("out_w", [48, 2], f32, False)
    out_b = nc.declare_dram_parameter("out_b", [2, 1], f32, False)
    inv_cnt = nc.declare_dram_parameter("inv_cnt", [96, G], f32, False)
    g1i = nc.declare_dram_parameter("g1", [128, TOT // 16], i16, False)
    g2i = nc.declare_dram_parameter("g2", [128, TOT // 16], i16, False)
    dstvi = nc.declare_dram_parameter("dstv", [128, TOT // 128], f16, False)
    s3i = nc.declare_dram_parameter("s3", [128, NSL // 16], i16, False)
    cmbi = nc.declare_dram_parameter("cmb", [128, 6], f32, False)
    wplacei = nc.declare_dram_parameter("wplace", [128, 8], i16, False)
    onehoti = nc.declare_dram_parameter("onehot", [NTL, 128, G], f16, False)
    yout = nc.declare_dram_parameter("y", [2, G], f32, True)

    htab = [nc.dram_tensor("htab0", [NC, HALF0, ROW], f16, addr_space="Shared"),
            nc.dram_tensor("htab1", [NC, HALF1, ROW], f16, addr_space="Shared")]
    own = [nc.dram_tensor("own0", [HALF0, ROW], f16),
           nc.dram_tensor("own1", [HALF1, ROW], f16)]
    stab = nc.dram_tensor("stab", [NSL, SROW], f16)
    padgrid = nc.dram_tensor("padgrid", [PAD3 + 128, HC], f32)
    maxgrid = nc.dram_tensor("maxgrid", [G + 1, HC], f32)
    poolsl = nc.dram_tensor("poolsl", [96, 4, G], f32)
    poolag = nc.dram_tensor("poolag", [NC, 96, 4, G], f32, addr_space="Shared")

    htabv = [htab[0][:].rearrange("c n e -> (c n) e"),
             htab[1][:].rearrange("c n e -> (c n) e")]

    with tile.TileContext(nc) as tc:
        with (
            tc.tile_pool(name="const", bufs=1) as constp,
            tc.tile_pool(name="wpool", bufs=1) as wpool,
            tc.tile_pool(name="lhs", bufs=3) as lhsp,
            tc.tile_pool(name="dense", bufs=3) as densep,
            tc.tile_pool(name="edge", bufs=3) as edgep,
            tc.tile_pool(name="edgec", bufs=2) as edgecp,
            tc.tile_pool(name="accp", bufs=1) as accp,
            tc.tile_pool(name="stagp", bufs=1) as stagp,
            tc.tile_pool(name="post", bufs=2) as postp,
            tc.tile_pool(name="xt", bufs=3) as xtp,
            tc.tile_pool(name="psum", bufs=2, space="PSUM") as psump,
            tc.tile_pool(name="psumP", bufs=1, space="PSUM") as psumPp,
            tc.tile_pool(name="small", bufs=1) as smallp,
        ):
            ident16 = constp.tile([128, 128], f16)
            make_identity(nc, ident16[:])
            identf = constp.tile([128, 128], f32)
            make_identity(nc, identf[:])
            iotai = constp.tile([128, 128], mybir.dt.int32)
            nc.gpsimd.iota(iotai[:], pattern=[[1, 128]], base=0,
                           channel_multiplier=0)
            iota16 = constp.tile([128, 128], f16)
            nc.vector.tensor_copy(iota16[:], iotai[:])
            eshift = constp.tile([128, 1], f32)
            nc.vector.memset(eshift[:], ESHIFT)
            zbias = constp.tile([128, 1], f32)
            nc.vector.memset(zbias[:], 0.0)

            wtA, wtB, bt = [], [], []
            for l in range(3):
                ka = FEAT if l == 0 else 96
                a = wpool.tile([ka, HC + 2 * H], f16, tag=f"wtA{l}")
                nc.sync.dma_start(a[:], Waug[l][:ka])
                wtA.append(a)
                if l == 0:
                    wtB.append(None)
                else:
                    b_ = wpool.tile([96, HC + 2 * H], f16, tag=f"wtB{l}")
                    nc.sync.dma_start(b_[:], Waug[l][96:])
                    wtB.append(b_)
                bb = wpool.tile([128, HC], f16, tag=f"bt{l}")
                nc.sync.dma_start(bb[:], brep[l][:])
                bt.append(bb)
            idxt = {}
            for nm, dram, w_ in (("g1", g1i, TOT // 16), ("g2", g2i, TOT // 16),
                                 ("s3", s3i, NSL // 16)):
                t = wpool.tile([128, w_], i16, tag=f"ix{nm}")
                nc.sync.dma_start(t[:], dram[:])
                idxt[nm] = t
            dstvt = wpool.tile([128, TOT // 128], f16, tag="dstv")
            nc.sync.dma_start(dstvt[:], dstvi[:])
            cmbt = wpool.tile([128, 6], f32, tag="cmb")
            nc.sync.dma_start(cmbt[:], cmbi[:])
            wplt = wpool.tile([128, 8], i16, tag="wpl")
            nc.sync.dma_start(wplt[:], wplacei[:])
            invt = wpool.tile([96, G], f32, tag="inv")
            nc.sync.dma_start(invt[:], inv_cnt[:])
            fc1wt = []
            for k in range(4):
                t = wpool.tile([96, 48], f32, tag=f"fc1{k}")
                nc.sync.dma_start(t[:], fc1_w[k * 96:(k + 1) * 96])
                fc1wt.append(t)
            fc1bt = wpool.tile([48, 1], f32, tag="fc1b")
            nc.sync.dma_start(fc1bt[:], fc1_b[:])
            outwt = wpool.tile([48, 2], f32, tag="outw")
            nc.sync.dma_start(outwt[:], out_w[:])
            outbt = wpool.tile([2, 1], f32, tag="outb")
            nc.sync.dma_start(outbt[:], out_b[:])

            acc = accp.tile([128, NTL, HC + H], f16, tag="acc")
            sdc = accp.tile([128, TOT // 128, H], f16, tag="sdc")
            stg_state = {"tile": None, "base": 0}
            stabst = stagp.tile([128, NTL, H], f16, tag="stabst")

            zt = constp.tile([128, 4, HC], f32, tag="zt")
            nc.vector.memset(zt[:], 0.0)
            r0 = 0
            while r0 < PAD3 + 128:
                rr = min(512, PAD3 + 128 - r0)
                nc.sync.dma_start(
                    padgrid[r0:r0 + rr].rearrange("(p a) c -> p (a c)", p=128),
                    zt[:, :rr // 128].rearrange("p a c -> p (a c)"))
                r0 += rr

            # ---- layer-1 stab (own s_dst rows from the per-core features)
            for t0 in range(0, NTL, 4):
                nb0 = min(4, NTL - t0)
                lhs = lhsp.tile([FEAT, 4 * 128], f16, tag="slhs")
                nc.sync.dma_start(lhs[:, :nb0 * 128],
                                  featTo[:, t0 * 128:(t0 + nb0) * 128])
                for j in range(nb0):
                    ps = psump.tile([128, H], f32, tag="dps")
                    nc.tensor.matmul(ps[:], lhs[:, j * 128:(j + 1) * 128],
                                     wtA[0][:, HC + H:], start=True, stop=True)
                    nc.scalar.activation(stabst[:, t0 + j], ps[:], AF.Copy)
            nc.sync.dma_start(
                stab[:].rearrange("(t p) c -> p t c", p=128)[:, :, :H],
                stabst[:])

            # ---- layer-1 dense-full: every core computes the whole htab
            BT = 8
            for half in (0, 1):
                ntile = NC * (T0 if half == 0 else T1)
                hv = htabv[half]
                for b0 in range(0, ntile, BT):
                    nb = min(BT, ntile - b0)
                    lhs = lhsp.tile([FEAT, BT * 128], f16, tag="dlhs")
                    nc.sync.dma_start(
                        lhs[:, :nb * 128],
                        featT[:, (half * NC * HALF0) + b0 * 128:
                              (half * NC * HALF0) + (b0 + nb) * 128])
                    hrow = densep.tile([128, BT, ROW], f16, tag="hrow")
                    for j in range(nb):
                        ps = psump.tile([128, HC + 2 * H], f32, tag="dps")
                        nc.tensor.matmul(ps[:], lhs[:, j * 128:(j + 1) * 128],
                                         wtA[0][:], start=True, stop=True)
                        nc.vector.tensor_copy(hrow[:, j, :HC + 2 * H], ps[:])
                    nc.scalar.dma_start(
                        hv[b0 * 128:(b0 + nb) * 128]
                        .rearrange("(j p) e -> p j e", p=128),
                        hrow[:, :nb])

            def post_tile(l, t, tmp):
                """tmp: SBUF fp32 [128, HC+H] aggregated messages + denoms."""
                den = postp.tile([128, H], f32, tag="pden")
                nc.vector.tensor_scalar(den[:], tmp[:, HC:], 1e-16, None,
                                        ALU.max)
                nc.vector.reciprocal(den[:], den[:])
                y16 = postp.tile([128, HC], f16, tag="py")
                for h in range(H):
                    nc.vector.tensor_scalar(
                        y16[:, h * C:(h + 1) * C], tmp[:, h * C:(h + 1) * C],
                        den[:, h:h + 1], None, ALU.mult)
                nc.vector.scalar_tensor_tensor(
                    y16[:], y16[:], 1.0, bt[l][:], ALU.bypass, ALU.add)
                e16 = postp.tile([128, HC], f16, tag="pe")
                nc.vector.tensor_scalar(e16[:], y16[:], 0.0, None, ALU.min)
                nc.scalar.activation(e16[:], e16[:], AF.Exp)
                nc.vector.scalar_tensor_tensor(
                    y16[:], e16[:], -1.0, y16[:], ALU.add, ALU.max)
                return y16

            def own_rows(l, t, y16):
                """Transpose y16 and compute next layer's own htab/stab rows."""
                xt = xtp.tile([96, 2, 128], f16, tag="xt")
                for blk in range(2):
                    pt = psump.tile([96, 128], f16, tag="tps")
                    nc.tensor.transpose(
                        pt[:], y16[:, blk * 96:(blk + 1) * 96], ident16[:])
                    nc.scalar.activation(xt[:, blk], pt[:], AF.Copy)
                ps2 = psump.tile([128, HC + 2 * H], f32, tag="dps")
                nc.tensor.matmul(ps2[:], xt[:, 0], wtA[l + 1][:],
                                 start=True, stop=False)
                nc.tensor.matmul(ps2[:], xt[:, 1], wtB[l + 1][:],
                                 start=False, stop=True)
                half = 0 if t < T0 else 1
                tl = t if t < T0 else t - T0
                if stg_state["tile"] is None:
                    stg_state["tile"] = stagp.tile([128, 4, ROW], f16,
                                                   tag="stg", name="stg")
                    stg_state["base"] = tl
                stg = stg_state["tile"]
                nc.scalar.activation(
                    stg[:, tl - stg_state["base"], :HC + 2 * H], ps2[:],
                    AF.Copy)
                nc.scalar.activation(stabst[:, t], ps2[:, HC + H:], AF.Copy)
                last_of_half = t == T0 - 1 or t == NTL - 1
                if tl - stg_state["base"] == 3 or last_of_half:
                    nch = tl - stg_state["base"] + 1
                    nc.sync.dma_start(
                        own[half][stg_state["base"] * 128:
                                  (stg_state["base"] + nch) * 128]
                        .rearrange("(t p) e -> p t e", p=128),
                        stg[:, :nch])
                    stg_state["tile"] = None
                if t == T0 - 1:
                    nc.gpsimd.collective_compute(
                        "AllGather", ALU.bypass, replica_groups=[core_ids],
                        ins=[own[0][:]], outs=[htab[0][:]])
                if t == NTL - 1:
                    nc.sync.dma_start(
                        stab[:].rearrange("(t p) c -> p t c", p=128)[:, :, :H],
                        stabst[:])

            wmax = stagp.tile([96, 2, NW], f32, tag="wmax")

            def wmax_tile(r):
                rows = postp.tile([128, HC], f32, tag="pyo")
                nc.sync.dma_start(rows[:], padgrid[r * 128:(r + 1) * 128])
                for blk in range(2):
                    pt = psump.tile([96, 128], f32, tag="tps")
                    nc.tensor.transpose(
                        pt[:], rows[:, blk * 96:(blk + 1) * 96], identf[:])
                    nc.vector.tensor_reduce(
                        wmax[:, blk, r * NPW:(r + 1) * NPW],
                        pt[:].rearrange("p (w q) -> p w q", q=WIN),
                        mybir.AxisListType.X, ALU.max)

            ohall = wpool.tile([128, NTL, G], f16, tag="ohall")
            nc.sync.dma_start(
                ohall[:], onehoti[:].rearrange("t p g -> p t g"))

            def pool_tile(t, y16, sump):
                for blk in range(2):
                    nc.tensor.matmul(
                        sump[blk][:], y16[:, blk * 96:(blk + 1) * 96],
                        ohall[:, t], start=(t == 0), stop=(t == NTL - 1))
                yo = postp.tile([128, HC], f32, tag="pyo")
                nc.vector.tensor_scalar(yo[:], y16[:], MAXOFF, None, ALU.add)
                nc.gpsimd.dma_scatter_add(
                    padgrid[int(lo_shared[t]):, :],
                    yo[:].rearrange("p (a c) -> p a c", a=1),
                    idxt["s3"][:, t * 8:(t + 1) * 8], 128, 128, HC,
                    single_packet=False)

            def edge_sd_group(h, tt, i0, S):
                cols = S // 128
                sd = edgep.tile([128, GC, SROW], f16, tag="sd")
                sbase = tt[0] * 128
                nc.gpsimd.dma_gather(
                    sd[:, :cols], stab[sbase:sbase + len(tt) * 128],
                    idxt["g2"][:, i0 // 16:(i0 + S) // 16], S, S, SROW,
                    single_packet=False, queue_num=1)
                nc.vector.tensor_copy(
                    sdc[:, i0 // 128:i0 // 128 + cols], sd[:, :cols, :H])

            def edge_sd_phase(l, half):
                for (h, tt, i0, S) in groups:
                    if h == half:
                        edge_sd_group(h, tt, i0, S)

            def edge_pass(l, half, sump, inline_sd=False):
                for (h, tt, i0, S) in groups:
                        if h != half:
                            continue
                        if inline_sd:
                            edge_sd_group(h, tt, i0, S)
                        cols = S // 128
                        big = edgep.tile([128, GC, ROW], f16, tag="big")
                        nc.gpsimd.dma_gather(
                            big[:, :cols], htabv[half],
                            idxt["g1"][:, i0 // 16:(i0 + S) // 16], S, S, ROW,
                            single_packet=False, queue_num=0)
                        u = edgecp.tile([128, GC, H], f32, tag="u")
                        nc.vector.scalar_tensor_tensor(
                            u[:, :cols], big[:, :cols, HC:HC + H], 1.0,
                            sdc[:, i0 // 128:i0 // 128 + cols],
                            ALU.bypass, ALU.add)
                        nc.vector.scalar_tensor_tensor(
                            u[:, :cols], u[:, :cols], NEG, u[:, :cols],
                            ALU.mult, ALU.max)
                        w4 = edgecp.tile([128, GC, H], f16, tag="w4")
                        nc.scalar.activation(w4[:, :cols], u[:, :cols],
                                             AF.Exp, bias=eshift[:])
                        WSUB = 8
                        for c0 in range(0, cols, WSUB):
                            cc = min(WSUB, cols - c0)
                            wfull = edgecp.tile([128, WSUB, HC], f16,
                                               tag="wfull")
                            nc.scalar.activation(
                                wfull[:, :cc].rearrange(
                                    "p c (h k) -> p c h k", h=H),
                                w4[:, c0:c0 + cc, :, None]
                                .to_broadcast([128, cc, H, C]), AF.Copy)
                            nc.vector.tensor_tensor(
                                big[:, c0:c0 + cc, :HC],
                                big[:, c0:c0 + cc, :HC],
                                wfull[:, :cc], ALU.mult)
                        nc.vector.tensor_copy(big[:, :cols, HC:HC + H],
                                              w4[:, :cols])
                        oh = edgecp.tile([128, GC, 128], f16, tag="oh")
                        nc.vector.tensor_tensor(
                            oh[:, :cols],
                            iota16[:, None, :].to_broadcast([128, cols, 128]),
                            dstvt[:, i0 // 128:i0 // 128 + cols, None]
                            .to_broadcast([128, cols, 128]),
                            ALU.is_equal)
                        q0 = 0
                        for t in tt:
                            ncq = int(bsz[half, t]) // 128
                            ps = psump.tile([128, HC + H], f32, tag="agg")
                            for q in range(ncq):
                                nc.tensor.matmul(
                                    ps[:], oh[:, q0 + q],
                                    big[:, q0 + q, :HC + H],
                                    start=(q == 0), stop=(q == ncq - 1))
                            q0 += ncq
                            if half == 0:
                                nc.scalar.activation(acc[:, t], ps[:], AF.Copy)
                            else:
                                tmp = postp.tile([128, HC + H], f32, tag="tmp")
                                nc.vector.scalar_tensor_tensor(
                                    tmp[:], ps[:], 1.0, acc[:, t],
                                    ALU.bypass, ALU.add)
                                y16 = post_tile(l, t, tmp)
                                if l < 2:
                                    own_rows(l, t, y16)
                                else:
                                    pool_tile(t, y16, sump)

            sump0 = psumPp.tile([96, G], f32, tag="sum0")
            sump1 = psumPp.tile([96, G], f32, tag="sum1")
            sump = [sump0, sump1]
            edge_pass(0, 0, None, inline_sd=True)
            for l in range(3):
                edge_pass(l, 1, sump if l == 2 else None,
                          inline_sd=(l == 0))
                if l < 2:
                    edge_sd_phase(l + 1, 0)
                    edge_pass(l + 1, 0, None)
                    edge_sd_phase(l + 1, 1)
                    nc.gpsimd.collective_compute(
                        "AllGather", ALU.bypass, replica_groups=[core_ids],
                        ins=[own[1][:]], outs=[htab[1][:]])

            # ---- pooling epilogue (fp32, as v1)
            for r in range(NT3):
                wmax_tile(r)
            wrow = smallp.tile([128, HC], f32, tag="wrow")
            for blk in range(2):
                pt2 = psump.tile([128, 96], f32, tag="tps")
                nc.tensor.transpose(pt2[:NW], wmax[:, blk],
                                    identf[:96, :96])
                nc.vector.tensor_copy(
                    wrow[:NW, blk * 96:(blk + 1) * 96], pt2[:NW])
            for ki, s in enumerate([1, 2, 4, 8, 16, 32]):
                if s >= NW:
                    break
                sh = smallp.tile([128, HC], f32, tag="wsh")
                nc.sync.dma_start(sh[:NW - s], wrow[s:NW])
                nc.vector.tensor_scalar(sh[:NW - s], sh[:NW - s],
                                        cmbt[:NW - s, ki:ki + 1],
                                        None, ALU.add)
                nc.vector.tensor_max(wrow[:NW - s], wrow[:NW - s],
                                     sh[:NW - s])
            zg = smallp.tile([G + 1, HC], f32, tag="zg")
            nc.vector.memset(zg[:], 0.0)
            nc.sync.dma_start(maxgrid[:], zg[:])
            nc.gpsimd.dma_scatter_add(
                maxgrid[:], wrow[:].rearrange("p (a c) -> p a c", a=1),
                wplt[:], 128, 128, HC, single_packet=False)
            mg = smallp.tile([G, HC], f32, tag="mg")
            nc.sync.dma_start(mg[:], maxgrid[:G])
            pp = smallp.tile([96, 4, G], f32, tag="pp")
            for blk in range(2):
                nc.vector.tensor_copy(pp[:, blk], sump0[:] if blk == 0
                                      else sump1[:])
                pt3 = psump.tile([96, G], f32, tag="tps")
                nc.tensor.transpose(
                    pt3[:], mg[:, blk * 96:(blk + 1) * 96],
                    identf[:G, :G])
                nc.vector.tensor_copy(pp[:, 2 + blk], pt3[:])
            nc.sync.dma_start(poolsl[:], pp[:])
            nc.gpsimd.collective_compute(
                "AllGather", ALU.bypass,
                replica_groups=[core_ids],
                ins=[poolsl[:]], outs=[poolag[:]])
            agg2 = smallp.tile([96, 4, G], f32, tag="agg2")
            for c_ in range(NC):
                at = smallp.tile([96, 4, G], f32, tag="agt")
                nc.sync.dma_start(at[:], poolag[c_])
                if c_ == 0:
                    nc.vector.tensor_copy(agg2[:], at[:])
                else:
                    nc.vector.tensor_add(agg2[:, :2], agg2[:, :2],
                                         at[:, :2])
                    nc.vector.tensor_max(agg2[:, 2:], agg2[:, 2:],
                                         at[:, 2:])
            for blk in range(2):
                nc.vector.tensor_mul(agg2[:, blk], agg2[:, blk], invt[:])
                nc.vector.tensor_scalar(agg2[:, 2 + blk],
                                        agg2[:, 2 + blk],
                                        -MAXOFF, None, ALU.add)
            zp = psump.tile([48, G], f32, tag="tps")
            for k in range(4):
                nc.tensor.matmul(zp[:], fc1wt[k], agg2[:, k],
                                 start=(k == 0), stop=(k == 3))
            z = smallp.tile([48, G], f32, tag="z")
            nc.vector.tensor_scalar(z[:], zp[:], fc1bt[:], None,
                                    ALU.add)
            e2 = smallp.tile([48, G], f32, tag="e2")
            nc.vector.tensor_scalar(e2[:], z[:], 0.0, None, ALU.min)
            nc.scalar.activation(e2[:], e2[:], AF.Exp)
            nc.vector.tensor_scalar(e2[:], e2[:], -1.0, None, ALU.add)
            nc.vector.tensor_max(z[:], z[:], e2[:])
            yp = psump.tile([2, G], f32, tag="tps")
            nc.tensor.matmul(yp[:], outwt[:], z[:], start=True,
                             stop=True)
            yf = smallp.tile([2, G], f32, tag="yf")
            nc.vector.tensor_scalar(yf[:], yp[:], outbt[:], None,
                                    ALU.add)
            nc.sync.dma_start(yout[:], yf[:])
    nc.finalize()
    return nc


def run(inputs, cfg, **run_kw):
    data, inv_cnt = host_prep(cfg, inputs["adj"], inputs["batch"])
    fl = prep_float_inputs(cfg, inputs)
    NC, NLOC, NSL = cfg["NC"], cfg["NLOC"], cfg["NSL"]
    in_maps = []
    for c in range(NC):
        m = dict(fl)
        m["inv_cnt"] = inv_cnt
        fto = np.zeros((cfg["FEAT"], NSL), np.float16)
        fto[:, :NLOC] = np.asarray(inputs["features"],
                                   np.float32)[c * NLOC:(c + 1) * NLOC].T
        m["featTown"] = fto
        m.update({k: data[c][k] for k in
                  ("g1", "g2", "dstv", "s3", "cmb", "wplace", "onehot")})
        in_maps.append(m)
    nc = build_program(cfg)
    from concourse.bass_utils import run_bass_kernel_spmd
    res = run_bass_kernel_spmd(nc, in_maps, list(range(NC)), **run_kw)
    y = np.asarray(res.results[0]["y"])
    return y.T.copy(), res


def kernel(**inputs):
    y, _ = run(inputs, make_cfg())
    return y


# revision 5
# speedup vs baseline: 1.0563x; 1.0085x over previous
"""3-layer GAT + graph pooling + MLP on 8 Trainium2 NeuronCores (Bass), v2.

Sharding: core c owns dst-nodes [c*NLOC, (c+1)*NLOC) and their in-edges.
All node tables are fp16; accumulation stays fp32 in PSUM.

Per layer:
  htab:   [h(192) | s_src(4) | s_dst(4) | pad] fp16 rows, 512B each, split in
          two half-tables (<=32768 rows each, int16 gather indices).  Layer 1
          computes it locally from the replicated features; layers 2-3
          AllGather per-core-computed own rows (h = x@Waug fused with the
          attention s columns), chunked in halves so the collective overlaps
          the previous layer's edge phase.
  edge:   two passes (one per half-table).  Per group of dst tiles:
          dma_gather 512B h-rows + 256B s_dst rows, w = exp(leaky(s)+shift)
          on ACT (fp16), scale h rows in place, one-hot via a single blocked
          is_equal, aggregate per 128-edge column on the TensorEngine:
          psum += onehot(dst)^T @ [w*h | w].  Pass 0 spills psum to an SBUF
          accumulator; pass 1 adds it back and runs post (softmax divide,
          bias, ELU) + transpose + the NEXT layer's own-row matmuls.
Pooling: as in v1 (unique-index scatter into graph-aligned slots, one-hot
sum pool, windowed max pool, small AllGather, replicated MLP).
All data-dependent structure (indices, one-hots) is INPUT DATA; the program
is static and identical across cores (SPMD).
"""

import sys
import numpy as np

sys.path.insert(0, "/opt/trn_rl_repo")

H, C = 4, 48
HC = H * C          # 192
NEG = 0.2
ESHIFT = -6.0       # global softmax shift: exp(leaky(u)+ESHIFT) fits fp16
BIGNEG = -2.0e30
MAXOFF = 1000.0     # max-pool offset: x3 = elu(...) >= -1, so x3+1000 > 0
ROW = 256           # htab row: h 192 | s_src 4 | s_dst 4 | pad (fp16)
SROW = 128          # stab row (fp16): s_dst 4 | pad  -> 256B
GROUP = 3           # dst tiles per gather call


def make_cfg(N=50000, E=800000, G=64, NC=8, FEAT=128, WIN=64):
    NLOC = N // NC
    assert NLOC * NC == N
    NTL = (NLOC + 127) // 128
    NSL = NTL * 128
    T0 = min((NTL + 1) // 2, max(1, int(NTL * 0.35)))
    T1 = NTL - T0
    assert T1 >= 1
    HALF0, HALF1 = T0 * 128, T1 * 128
    assert NC * HALF0 <= 32768 and NC * HALF1 <= 32768
    return dict(N=N, E=E, G=G, NC=NC, FEAT=FEAT, NLOC=NLOC, NSL=NSL, NTL=NTL,
                T0=T0, T1=T1, HALF0=HALF0, HALF1=HALF1, GSL=NC * NSL, WIN=WIN)


def _wrap_idx(idx):
    """SWDGE idx layout: element i -> [i % 16, i // 16], replicated to 128
    partitions (one copy per Q7 core)."""
    T = idx.shape[0]
    out = np.ascontiguousarray(idx.reshape(T // 16, 16).T).astype(np.int16)
    return np.tile(out, (8, 1))


def host_prep(cfg, adj, batch):
    N, G, NC = cfg["N"], cfg["G"], cfg["NC"]
    NLOC, NTL, WIN = cfg["NLOC"], cfg["NTL"], cfg["WIN"]
    NSL, T0, HALF0, HALF1 = cfg["NSL"], cfg["T0"], cfg["HALF0"], cfg["HALF1"]
    src = np.asarray(adj[0], dtype=np.int64)
    dst = np.asarray(adj[1], dtype=np.int64)
    batch = np.asarray(batch, dtype=np.int64)
    c_src = src // NLOC
    n_src = src % NLOC
    half_src = (n_src >= HALF0).astype(np.int64)
    row_src = np.where(half_src == 0, c_src * HALF0 + n_src,
                       c_src * HALF1 + (n_src - HALF0))
    counts_g = np.bincount(batch, minlength=G)

    # ---- edge grouping: (half, dst-tile) blocks, each padded %128, >=128
    blocks_all = []
    sizes = np.zeros((NC, 2, NTL), dtype=np.int64)
    for c in range(NC):
        lo = c * NLOC
        esel = np.nonzero((dst >= lo) & (dst < lo + NLOC))[0]
        dt_of = (dst[esel] - lo) // 128
        hf_of = half_src[esel]
        d = {}
        for h in range(2):
            for t in range(NTL):
                ee = esel[(hf_of == h) & (dt_of == t)]
                d[(h, t)] = ee
                sizes[c, h, t] = len(ee)
        blocks_all.append(d)
    bsz = np.zeros((2, NTL), dtype=np.int64)
    for h in range(2):
        for t in range(NTL):
            m = max(int(sizes[:, h, t].max()), 1)
            bsz[h, t] = -(-m // 128) * 128
    offs = np.zeros((2, NTL), dtype=np.int64)
    o = 0
    for h in range(2):
        for t in range(NTL):
            offs[h, t] = o
            o += bsz[h, t]
    TOT = int(o)

    # gather groups: (half, [tiles], i0, S) — every dst tile has a block in
    # each src half-table; greedy-pack tiles up to a column cap
    CAP = 30 * 128
    groups = []
    for h in range(2):
        tt, S = [], 0
        for t in range(NTL):
            b = int(bsz[h, t])
            if tt and S + b > CAP:
                groups.append((h, list(tt), int(offs[h, tt[0]]), S))
                tt, S = [], 0
            tt.append(t)
            S += b
        if tt:
            groups.append((h, list(tt), int(offs[h, tt[0]]), S))
    GC = max(S // 128 for (_, _, _, S) in groups)

    # ---- layer-3 graph-aligned slots (as v1)
    pad3_meta, pad3_tot = [], 0
    for c in range(NC):
        lo = c * NLOC
        b = batch[lo:lo + NLOC]
        gids, starts = np.unique(b, return_index=True)
        osort = np.argsort(starts)
        gids, starts = gids[osort], starts[osort]
        ends = np.append(starts[1:], NLOC)
        slots = np.empty(NLOC, dtype=np.int64)
        wg, fwin = [], []
        pos = 0
        for g, s, e in zip(gids, starts, ends):
            cnt = e - s
            slots[s:e] = pos + np.arange(cnt)
            nw = -(-cnt // WIN)
            wg += [int(g)] * nw
            fwin += [1] + [0] * (nw - 1)
            pos += nw * WIN
        pad3_meta.append((slots, wg, fwin))
        pad3_tot = max(pad3_tot, pos)
    PAD3 = -(-pad3_tot // 128) * 128
    NW, NT3 = PAD3 // WIN, PAD3 // 128
    assert NW <= 128
    # shared (min over cores) scatter lower bound per dst tile, for sliced
    # scatter APs so pooling window reductions can start before all tiles land
    lo_shared = np.full(NTL, 10**9, dtype=np.int64)
    for c in range(NC):
        slots3 = pad3_meta[c][0]
        for t in range(NTL):
            sl = slots3[t * 128:min((t + 1) * 128, NLOC)]
            if len(sl):
                lo_shared[t] = min(lo_shared[t], int(sl.min()))
    lo_shared = np.minimum.accumulate(lo_shared[::-1])[::-1]
    lo_shared[0] = 0
    rmap = np.zeros(NT3, dtype=np.int64)
    for r in range(NT3):
        sel = np.nonzero(lo_shared <= (r + 1) * 128 - 1)[0]
        rmap[r] = int(sel.max()) if len(sel) else 0
    cfg.update(TOT=TOT, bsz=bsz, offs=offs, groups=groups, GC=GC,
               PAD3=PAD3, NW=NW, NT3=NT3, lo_shared=lo_shared, rmap=rmap)

    gbase = {}
    for (h, tt, i0, S) in groups:
        for t in tt:
            gbase[(h, t)] = tt[0]

    data = []
    strides = [1, 2, 4, 8, 16, 32]
    for c in range(NC):
        lo = c * NLOC
        slots3, wg, fwin = pad3_meta[c]
        g1 = np.zeros(TOT, dtype=np.int64)
        g2 = np.zeros(TOT, dtype=np.int64)
        dstv = np.full(TOT, 999.0, dtype=np.float16)
        for h in range(2):
            for t in range(NTL):
                ee = blocks_all[c][(h, t)]
                i0 = int(offs[h, t])
                g1[i0:i0 + len(ee)] = row_src[ee]
                g2[i0:i0 + len(ee)] = (dst[ee] - lo) - gbase[(h, t)] * 128
                dstv[i0:i0 + len(ee)] = ((dst[ee] - lo) % 128).astype(np.float16)
        assert g1.min() >= 0 and g2.min() >= 0
        s3 = np.full(NSL, PAD3, dtype=np.int64)
        s3[:NLOC] = slots3
        for t in range(NTL):
            s3[t * 128:(t + 1) * 128] -= lo_shared[t]
        assert s3.min() >= 0
        wgp = np.full(NW, -1, dtype=np.int64)
        wgp[:len(wg)] = wg
        cmb = np.full((128, len(strides)), BIGNEG, dtype=np.float32)
        for k, s in enumerate(strides):
            for i in range(NW - s):
                if wgp[i] >= 0 and wgp[i] == wgp[i + s]:
                    cmb[i, k] = 0.0
        wplace = np.full(128, G, dtype=np.int64)
        for i in range(len(wg)):
            if fwin[i]:
                wplace[i] = wg[i]
        onehot = np.zeros((NTL, 128, G), dtype=np.float16)
        nn = np.arange(NLOC)
        onehot[nn // 128, nn % 128, batch[lo:lo + NLOC]] = 1.0
        data.append(dict(
            g1=_wrap_idx(g1),
            g2=_wrap_idx(g2),
            dstv=np.ascontiguousarray(
                dstv.reshape(TOT // 128, 128).T).astype(np.float16),
            s3=_wrap_idx(s3),
            cmb=cmb,
            wplace=_wrap_idx(wplace),
            onehot=onehot,
        ))
    inv_cnt = np.tile((1.0 / np.maximum(counts_g, 1.0))
                      .astype(np.float32)[None, :], (96, 1))
    return data, inv_cnt


def prep_float_inputs(cfg, inputs):
    NC, NLOC, NSL, FEAT = cfg["NC"], cfg["NLOC"], cfg["NSL"], cfg["FEAT"]
    HALF0, HALF1, GSL = cfg["HALF0"], cfg["HALF1"], cfg["GSL"]
    f = {}
    feat = np.asarray(inputs["features"], np.float32)
    fpad = np.zeros((NC, NSL, FEAT), np.float32)
    for c in range(NC):
        fpad[c, :NLOC] = feat[c * NLOC:(c + 1) * NLOC]
    h0 = fpad[:, :HALF0].reshape(NC * HALF0, FEAT)
    h1 = fpad[:, HALF0:].reshape(NC * HALF1, FEAT)
    f["featT"] = np.ascontiguousarray(
        np.concatenate([h0, h1], axis=0).T).astype(np.float16)
    for l in (1, 2, 3):
        W = np.asarray(inputs[f"W{l}"], np.float32)
        A = np.zeros((HC, 2 * H), np.float32)
        for h in range(H):
            A[h * C:(h + 1) * C, h] = np.asarray(inputs[f"a_src{l}"], np.float32)[h]
            A[h * C:(h + 1) * C, H + h] = np.asarray(inputs[f"a_dst{l}"], np.float32)[h]
        f[f"Waug{l}"] = np.concatenate([W, W @ A], axis=1).astype(np.float16)
        f[f"brep{l}"] = np.tile(np.asarray(inputs[f"b{l}"], np.float16)[None, :],
                                (128, 1))
    f["fc1_w"] = np.asarray(inputs["fc1_w"], np.float32)
    f["fc1_b"] = np.asarray(inputs["fc1_b"], np.float32).reshape(-1, 1)
    f["out_w"] = np.asarray(inputs["out_w"], np.float32)
    f["out_b"] = np.asarray(inputs["out_b"], np.float32).reshape(-1, 1)
    return f


def build_program(cfg):
    from concourse import bacc, bass, mybir, tile
    from concourse.masks import make_identity
    f32, f16, i16 = mybir.dt.float32, mybir.dt.float16, mybir.dt.int16
    AF, ALU = mybir.ActivationFunctionType, mybir.AluOpType
    G, NC, FEAT = cfg["G"], cfg["NC"], cfg["FEAT"]
    NLOC, NSL, NTL, GSL = cfg["NLOC"], cfg["NSL"], cfg["NTL"], cfg["GSL"]
    T0, T1, HALF0, HALF1 = cfg["T0"], cfg["T1"], cfg["HALF0"], cfg["HALF1"]
    TOT, PAD3 = cfg["TOT"], cfg["PAD3"]
    NW, NT3, WIN = cfg["NW"], cfg["NT3"], cfg["WIN"]
    NPW = 128 // WIN
    bsz, offs, groups, GC = cfg["bsz"], cfg["offs"], cfg["groups"], cfg["GC"]
    lo_shared, rmap = cfg["lo_shared"], cfg["rmap"]
    core_ids = list(range(NC))

    nc = bacc.Bacc(None, num_devices=NC, num_swdge_queues=2)

    featT = nc.declare_dram_parameter("featT", [FEAT, GSL], f16, False)
    featTo = nc.declare_dram_parameter("featTown", [FEAT, NSL], f16, False)
    Waug, brep = [], []
    for l in (1, 2, 3):
        Waug.append(nc.declare_dram_parameter(
            f"Waug{l}", [FEAT if l == 1 else HC, HC + 2 * H], f16, False))
        brep.append(nc.declare_dram_parameter(f"brep{l}", [128, HC], f16, False))
    fc1_w = nc.declare_dram_parameter("fc1_w", [2 * HC, 48], f32, False)
    fc1_b = nc.declare_dram_parameter("fc1_b", [48, 1], f32, False)
    out_w = nc.declare_dram_parameter("out_w", [48, 2], f32, False)
    out_b = nc.declare_dram_parameter("out_b", [2, 1], f32, False)
    inv_cnt = nc.declare_dram_parameter("inv_cnt", [96, G], f32, False)
    g1i = nc.declare_dram_parameter("g1", [128, TOT // 16], i16, False)
    g2i = nc.declare_dram_parameter("g2", [128, TOT // 16], i16, False)
    dstvi = nc.declare_dram_parameter("dstv", [128, TOT // 128], f16, False)
    s3i = nc.declare_dram_parameter("s3", [128, NSL // 16], i16, False)
    cmbi = nc.declare_dram_parameter("cmb", [128, 6], f32, False)
    wplacei = nc.declare_dram_parameter("wplace", [128, 8], i16, False)
    onehoti = nc.declare_dram_parameter("onehot", [NTL, 128, G], f16, False)
    yout = nc.declare_dram_parameter("y", [2, G], f32, True)

    htab = [nc.dram_tensor("htab0", [NC, HALF0, ROW], f16, addr_space="Shared"),
            nc.dram_tensor("htab1", [NC, HALF1, ROW], f16, addr_space="Shared")]
    own = [nc.dram_tensor("own0", [HALF0, ROW], f16),
           nc.dram_tensor("own1", [HALF1, ROW], f16)]
    stab = nc.dram_tensor("stab", [NSL, SROW], f16)
    padgrid = nc.dram_tensor("padgrid", [PAD3 + 128, HC], f32)
    maxgrid = nc.dram_tensor("maxgrid", [G + 1, HC], f32)
    poolsl = nc.dram_tensor("poolsl", [96, 4, G], f32)
    poolag = nc.dram_tensor("poolag", [NC, 96, 4, G], f32, addr_space="Shared")

    htabv = [htab[0][:].rearrange("c n e -> (c n) e"),
             htab[1][:].rearrange("c n e -> (c n) e")]

    with tile.TileContext(nc) as tc:
        with (
            tc.tile_pool(name="const", bufs=1) as constp,
            tc.tile_pool(name="wpool", bufs=1) as wpool,
            tc.tile_pool(name="lhs", bufs=3) as lhsp,
            tc.tile_pool(name="dense", bufs=3) as densep,
            tc.tile_pool(name="edge", bufs=3) as edgep,
            tc.tile_pool(name="edgec", bufs=2) as edgecp,
            tc.tile_pool(name="accp", bufs=1) as accp,
            tc.tile_pool(name="stagp", bufs=1) as stagp,
            tc.tile_pool(name="post", bufs=2) as postp,
            tc.tile_pool(name="xt", bufs=3) as xtp,
            tc.tile_pool(name="psum", bufs=2, space="PSUM") as psump,
            tc.tile_pool(name="psumP", bufs=1, space="PSUM") as psumPp,
            tc.tile_pool(name="small", bufs=1) as smallp,
        ):
            ident16 = constp.tile([128, 128], f16)
            make_identity(nc, ident16[:])
            identf = constp.tile([128, 128], f32)
            make_identity(nc, identf[:])
            iotai = constp.tile([128, 128], mybir.dt.int32)
            nc.gpsimd.iota(iotai[:], pattern=[[1, 128]], base=0,
                           channel_multiplier=0)
            iota16 = constp.tile([128, 128], f16)
            nc.vector.tensor_copy(iota16[:], iotai[:])
            eshift = constp.tile([128, 1], f32)
            nc.vector.memset(eshift[:], ESHIFT)
            zbias = constp.tile([128, 1], f32)
            nc.vector.memset(zbias[:], 0.0)

            wtA, wtB, bt = [], [], []
            for l in range(3):
                ka = FEAT if l == 0 else 96
                a = wpool.tile([ka, HC + 2 * H], f16, tag=f"wtA{l}")
                nc.sync.dma_start(a[:], Waug[l][:ka])
                wtA.append(a)
                if l == 0:
                    wtB.append(None)
                else:
                    b_ = wpool.tile([96, HC + 2 * H], f16, tag=f"wtB{l}")
                    nc.sync.dma_start(b_[:], Waug[l][96:])
                    wtB.append(b_)
                bb = wpool.tile([128, HC], f16, tag=f"bt{l}")
                nc.sync.dma_start(bb[:], brep[l][:])
                bt.append(bb)
            idxt = {}
            for nm, dram, w_ in (("g1", g1i, TOT // 16), ("g2", g2i, TOT // 16),
                                 ("s3", s3i, NSL // 16)):
                t = wpool.tile([128, w_], i16, tag=f"ix{nm}")
                nc.sync.dma_start(t[:], dram[:])
                idxt[nm] = t
            dstvt = wpool.tile([128, TOT // 128], f16, tag="dstv")
            nc.sync.dma_start(dstvt[:], dstvi[:])
            cmbt = wpool.tile([128, 6], f32, tag="cmb")
            nc.sync.dma_start(cmbt[:], cmbi[:])
            wplt = wpool.tile([128, 8], i16, tag="wpl")
            nc.sync.dma_start(wplt[:], wplacei[:])
            invt = wpool.tile([96, G], f32, tag="inv")
            nc.sync.dma_start(invt[:], inv_cnt[:])
            fc1wt = []
            for k in range(4):
                t = wpool.tile([96, 48], f32, tag=f"fc1{k}")
                nc.sync.dma_start(t[:], fc1_w[k * 96:(k + 1) * 96])
                fc1wt.append(t)
            fc1bt = wpool.tile([48, 1], f32, tag="fc1b")
            nc.sync.dma_start(fc1bt[:], fc1_b[:])
            outwt = wpool.tile([48, 2], f32, tag="outw")
            nc.sync.dma_start(outwt[:], out_w[:])
            outbt = wpool.tile([2, 1], f32, tag="outb")
            nc.sync.dma_start(outbt[:], out_b[:])

            acc = accp.tile([128, NTL, HC + H], f16, tag="acc")
            sdc = accp.tile([128, TOT // 128, H], f16, tag="sdc")
            stg_state = {"tile": None, "base": 0}
            stabst = stagp.tile([128, NTL, H], f16, tag="stabst")

            zt = constp.tile([128, 4, HC], f32, tag="zt")
            nc.vector.memset(zt[:], 0.0)
            r0 = 0
            while r0 < PAD3 + 128:
                rr = min(512, PAD3 + 128 - r0)
                nc.sync.dma_start(
                    padgrid[r0:r0 + rr].rearrange("(p a) c -> p (a c)", p=128),
                    zt[:, :rr // 128].rearrange("p a c -> p (a c)"))
                r0 += rr

            # ---- layer-1 stab (own s_dst rows from the per-core features)
            for t0 in range(0, NTL, 4):
                nb0 = min(4, NTL - t0)
                lhs = lhsp.tile([FEAT, 4 * 128], f16, tag="slhs")
                nc.sync.dma_start(lhs[:, :nb0 * 128],
                                  featTo[:, t0 * 128:(t0 + nb0) * 128])
                for j in range(nb0):
                    ps = psump.tile([128, H], f32, tag="dps")
                    nc.tensor.matmul(ps[:], lhs[:, j * 128:(j + 1) * 128],
                                     wtA[0][:, HC + H:], start=True, stop=True)
                    nc.scalar.activation(stabst[:, t0 + j], ps[:], AF.Copy)
            nc.sync.dma_start(
                stab[:].rearrange("(t p) c -> p t c", p=128)[:, :, :H],
                stabst[:])

            # ---- layer-1 dense-full: every core computes the whole htab
            BT = 8
            for half in (0, 1):
                ntile = NC * (T0 if half == 0 else T1)
                hv = htabv[half]
                for b0 in range(0, ntile, BT):
                    nb = min(BT, ntile - b0)
                    lhs = lhsp.tile([FEAT, BT * 128], f16, tag="dlhs")
                    nc.sync.dma_start(
                        lhs[:, :nb * 128],
                        featT[:, (half * NC * HALF0) + b0 * 128:
                              (half * NC * HALF0) + (b0 + nb) * 128])
                    hrow = densep.tile([128, BT, ROW], f16, tag="hrow")
                    for j in range(nb):
                        ps = psump.tile([128, HC + 2 * H], f32, tag="dps")
                        nc.tensor.matmul(ps[:], lhs[:, j * 128:(j + 1) * 128],
                                         wtA[0][:], start=True, stop=True)
                        nc.vector.tensor_copy(hrow[:, j, :HC + 2 * H], ps[:])
                    nc.scalar.dma_start(
                        hv[b0 * 128:(b0 + nb) * 128]
                        .rearrange("(j p) e -> p j e", p=128),
                        hrow[:, :nb])

            def post_tile(l, t, tmp):
                """tmp: SBUF fp32 [128, HC+H] aggregated messages + denoms."""
                den = postp.tile([128, H], f32, tag="pden")
                nc.vector.tensor_scalar(den[:], tmp[:, HC:], 1e-16, None,
                                        ALU.max)
                nc.vector.reciprocal(den[:], den[:])
                y16 = postp.tile([128, HC], f16, tag="py")
                for h in range(H):
                    nc.vector.tensor_scalar(
                        y16[:, h * C:(h + 1) * C], tmp[:, h * C:(h + 1) * C],
                        den[:, h:h + 1], None, ALU.mult)
                nc.vector.scalar_tensor_tensor(
                    y16[:], y16[:], 1.0, bt[l][:], ALU.bypass, ALU.add)
                e16 = postp.tile([128, HC], f16, tag="pe")
                nc.vector.tensor_scalar(e16[:], y16[:], 0.0, None, ALU.min)
                nc.scalar.activation(e16[:], e16[:], AF.Exp)
                nc.vector.scalar_tensor_tensor(
                    y16[:], e16[:], -1.0, y16[:], ALU.add, ALU.max)
                return y16

            def own_rows(l, t, y16):
                """Transpose y16 and compute next layer's own htab/stab rows."""
                xt = xtp.tile([96, 2, 128], f16, tag="xt")
                for blk in range(2):
                    pt = psump.tile([96, 128], f16, tag="tps")
                    nc.tensor.transpose(
                        pt[:], y16[:, blk * 96:(blk + 1) * 96], ident16[:])
                    nc.scalar.activation(xt[:, blk], pt[:], AF.Copy)
                ps2 = psump.tile([128, HC + 2 * H], f32, tag="dps")
                nc.tensor.matmul(ps2[:], xt[:, 0], wtA[l + 1][:],
                                 start=True, stop=False)
                nc.tensor.matmul(ps2[:], xt[:, 1], wtB[l + 1][:],
                                 start=False, stop=True)
                half = 0 if t < T0 else 1
                tl = t if t < T0 else t - T0
                if stg_state["tile"] is None:
                    stg_state["tile"] = stagp.tile([128, 4, ROW], f16,
                                                   tag="stg", name="stg")
                    stg_state["base"] = tl
                stg = stg_state["tile"]
                nc.scalar.activation(
                    stg[:, tl - stg_state["base"], :HC + 2 * H], ps2[:],
                    AF.Copy)
                nc.scalar.activation(stabst[:, t], ps2[:, HC + H:], AF.Copy)
                last_of_half = t == T0 - 1 or t == NTL - 1
                if tl - stg_state["base"] == 3 or last_of_half:
                    nch = tl - stg_state["base"] + 1
                    nc.sync.dma_start(
                        own[half][stg_state["base"] * 128:
                                  (stg_state["base"] + nch) * 128]
                        .rearrange("(t p) e -> p t e", p=128),
                        stg[:, :nch])
                    stg_state["tile"] = None
                if t == T0 - 1:
                    nc.gpsimd.collective_compute(
                        "AllGather", ALU.bypass, replica_groups=[core_ids],
                        ins=[own[0][:]], outs=[htab[0][:]])
                if t == NTL - 1:
                    nc.sync.dma_start(
                        stab[:].rearrange("(t p) c -> p t c", p=128)[:, :, :H],
                        stabst[:])

            wmax = stagp.tile([96, 2, NW], f32, tag="wmax")

            def wmax_tile(r):
                rows = postp.tile([128, HC], f32, tag="pyo")
                nc.sync.dma_start(rows[:], padgrid[r * 128:(r + 1) * 128])
                for blk in range(2):
                    pt = psump.tile([96, 128], f32, tag="tps")
                    nc.tensor.transpose(
                        pt[:], rows[:, blk * 96:(blk + 1) * 96], identf[:])
                    nc.vector.tensor_reduce(
                        wmax[:, blk, r * NPW:(r + 1) * NPW],
                        pt[:].rearrange("p (w q) -> p w q", q=WIN),
                        mybir.AxisListType.X, ALU.max)

            ohall = wpool.tile([128, NTL, G], f16, tag="ohall")
            nc.sync.dma_start(
                ohall[:], onehoti[:].rearrange("t p g -> p t g"))

            def pool_tile(t, y16, sump):
                for blk in range(2):
                    nc.tensor.matmul(
                        sump[blk][:], y16[:, blk * 96:(blk + 1) * 96],
                        ohall[:, t], start=(t == 0), stop=(t == NTL - 1))
                yo = postp.tile([128, HC], f32, tag="pyo")
                nc.vector.tensor_scalar(yo[:], y16[:], MAXOFF, None, ALU.add)
                nc.gpsimd.dma_scatter_add(
                    padgrid[int(lo_shared[t]):, :],
                    yo[:].rearrange("p (a c) -> p a c", a=1),
                    idxt["s3"][:, t * 8:(t + 1) * 8], 128, 128, HC,
                    single_packet=False)

            def edge_sd_group(h, tt, i0, S):
                cols = S // 128
                sd = edgep.tile([128, GC, SROW], f16, tag="sd")
                sbase = tt[0] * 128
                nc.gpsimd.dma_gather(
                    sd[:, :cols], stab[sbase:sbase + len(tt) * 128],
                    idxt["g2"][:, i0 // 16:(i0 + S) // 16], S, S, SROW,
                    single_packet=False, queue_num=1)
                nc.vector.tensor_copy(
                    sdc[:, i0 // 128:i0 // 128 + cols], sd[:, :cols, :H])

            def edge_sd_phase(l, half):
                for (h, tt, i0, S) in groups:
                    if h == half:
                        edge_sd_group(h, tt, i0, S)

            def edge_pass(l, half, sump, inline_sd=False):
                for (h, tt, i0, S) in groups:
                        if h != half:
                            continue
                        if inline_sd:
                            edge_sd_group(h, tt, i0, S)
                        cols = S // 128
                        big = edgep.tile([128, GC, ROW], f16, tag="big")
                        nc.gpsimd.dma_gather(
                            big[:, :cols], htabv[half],
                            idxt["g1"][:, i0 // 16:(i0 + S) // 16], S, S, ROW,
                            single_packet=False, queue_num=0)
                        u = edgecp.tile([128, GC, H], f32, tag="u")
                        nc.vector.scalar_tensor_tensor(
                            u[:, :cols], big[:, :cols, HC:HC + H], 1.0,
                            sdc[:, i0 // 128:i0 // 128 + cols],
                            ALU.bypass, ALU.add)
                        nc.vector.scalar_tensor_tensor(
                            u[:, :cols], u[:, :cols], NEG, u[:, :cols],
                            ALU.mult, ALU.max)
                        w4 = edgecp.tile([128, GC, H], f16, tag="w4")
                        nc.scalar.activation(w4[:, :cols], u[:, :cols],
                                             AF.Exp, bias=eshift[:])
                        WSUB = 8
                        for c0 in range(0, cols, WSUB):
                            cc = min(WSUB, cols - c0)
                            wfull = edgecp.tile([128, WSUB, HC], f16,
                                               tag="wfull")
                            nc.scalar.activation(
                                wfull[:, :cc].rearrange(
                                    "p c (h k) -> p c h k", h=H),
                                w4[:, c0:c0 + cc, :, None]
                                .to_broadcast([128, cc, H, C]), AF.Copy)
                            nc.vector.tensor_tensor(
                                big[:, c0:c0 + cc, :HC],
                                big[:, c0:c0 + cc, :HC],
                                wfull[:, :cc], ALU.mult)
                        nc.vector.tensor_copy(big[:, :cols, HC:HC + H],
                                              w4[:, :cols])
                        oh = edgecp.tile([128, GC, 128], f16, tag="oh")
                        nc.vector.tensor_tensor(
                            oh[:, :cols],
                            iota16[:, None, :].to_broadcast([128, cols, 128]),
                            dstvt[:, i0 // 128:i0 // 128 + cols, None]
                            .to_broadcast([128, cols, 128]),
                            ALU.is_equal)
                        q0 = 0
                        for t in tt:
                            ncq = int(bsz[half, t]) // 128
                            ps = psump.tile([128, HC + H], f32, tag="agg")
                            for q in range(ncq):
                                nc.tensor.matmul(
                                    ps[:], oh[:, q0 + q],
                                    big[:, q0 + q, :HC + H],
                                    start=(q == 0), stop=(q == ncq - 1))
                            q0 += ncq
                            if half == 0:
                                nc.scalar.activation(acc[:, t], ps[:], AF.Copy)
                            else:
                                tmp = postp.tile([128, HC + H], f32, tag="tmp")
                                nc.vector.scalar_tensor_tensor(
                                    tmp[:], ps[:], 1.0, acc[:, t],
                                    ALU.bypass, ALU.add)
                                y16 = post_tile(l, t, tmp)
                                if l < 2:
                                    own_rows(l, t, y16)
                                else:
                                    pool_tile(t, y16, sump)

            sump0 = psumPp.tile([96, G], f32, tag="sum0")
            sump1 = psumPp.tile([96, G], f32, tag="sum1")
            sump = [sump0, sump1]
            edge_pass(0, 0, None, inline_sd=True)
            for l in range(3):
                edge_pass(l, 1, sump if l == 2 else None,
                          inline_sd=(l == 0))
                if l < 2:
                    edge_sd_phase(l + 1, 0)
                    edge_pass(l + 1, 0, None)
                    edge_sd_phase(l + 1, 1)
                    nc.gpsimd.collective_compute(
                        "AllGather", ALU.bypass, replica_groups=[core_ids],
                        ins=[own[1][:]], outs=[htab[1][:]])

            # ---- pooling epilogue (fp32, as v1)
            for r in range(NT3):
                wmax_tile(r)
            wrow = smallp.tile([128, HC], f32, tag="wrow")
            for blk in range(2):
                pt2 = psump.tile([128, 96], f32, tag="tps")
                nc.tensor.transpose(pt2[:NW], wmax[:, blk],
                                    identf[:96, :96])
                nc.vector.tensor_copy(
                    wrow[:NW, blk * 96:(blk + 1) * 96], pt2[:NW])
            for ki, s in enumerate([1, 2, 4, 8, 16, 32]):
                if s >= NW:
                    break
                sh = smallp.tile([128, HC], f32, tag="wsh")
                nc.sync.dma_start(sh[:NW - s], wrow[s:NW])
                nc.vector.tensor_scalar(sh[:NW - s], sh[:NW - s],
                                        cmbt[:NW - s, ki:ki + 1],
                                        None, ALU.add)
                nc.vector.tensor_max(wrow[:NW - s], wrow[:NW - s],
                                     sh[:NW - s])
            zg = smallp.tile([G + 1, HC], f32, tag="zg")
            nc.vector.memset(zg[:], 0.0)
            nc.sync.dma_start(maxgrid[:], zg[:])
            nc.gpsimd.dma_scatter_add(
                maxgrid[:], wrow[:].rearrange("p (a c) -> p a c", a=1),
                wplt[:], 128, 128, HC, single_packet=False)
            mg = smallp.tile([G, HC], f32, tag="mg")
            nc.sync.dma_start(mg[:], maxgrid[:G])
            pp = smallp.tile([96, 4, G], f32, tag="pp")
            for blk in range(2):
                nc.vector.tensor_copy(pp[:, blk], sump0[:] if blk == 0
                                      else sump1[:])
                pt3 = psump.tile([96, G], f32, tag="tps")
                nc.tensor.transpose(
                    pt3[:], mg[:, blk * 96:(blk + 1) * 96],
                    identf[:G, :G])
                nc.vector.tensor_copy(pp[:, 2 + blk], pt3[:])
            nc.sync.dma_start(poolsl[:], pp[:])
            nc.gpsimd.collective_compute(
                "AllGather", ALU.bypass,
                replica_groups=[core_ids],
                ins=[poolsl[:]], outs=[poolag[:]])
            agg2 = smallp.tile([96, 4, G], f32, tag="agg2")
            at = smallp.tile([96, NC, 4, G], f32, tag="agt")
            nc.sync.dma_start(at[:], poolag[:].rearrange("c p f g -> p c f g"))
            nc.vector.tensor_copy(agg2[:], at[:, 0])
            for c_ in range(1, NC):
                nc.vector.tensor_add(agg2[:, :2], agg2[:, :2],
                                     at[:, c_, :2])
                nc.vector.tensor_max(agg2[:, 2:], agg2[:, 2:],
                                     at[:, c_, 2:])
            for blk in range(2):
                nc.vector.tensor_mul(agg2[:, blk], agg2[:, blk], invt[:])
                nc.vector.tensor_scalar(agg2[:, 2 + blk],
                                        agg2[:, 2 + blk],
                                        -MAXOFF, None, ALU.add)
            zp = psump.tile([48, G], f32, tag="tps")
            for k in range(4):
                nc.tensor.matmul(zp[:], fc1wt[k], agg2[:, k],
                                 start=(k == 0), stop=(k == 3))
            z = smallp.tile([48, G], f32, tag="z")
            nc.vector.tensor_scalar(z[:], zp[:], fc1bt[:], None,
                                    ALU.add)
            e2 = smallp.tile([48, G], f32, tag="e2")
            nc.vector.tensor_scalar(e2[:], z[:], 0.0, None, ALU.min)
            nc.scalar.activation(e2[:], e2[:], AF.Exp)
            nc.vector.tensor_scalar(e2[:], e2[:], -1.0, None, ALU.add)
            nc.vector.tensor_max(z[:], z[:], e2[:])
            yp = psump.tile([2, G], f32, tag="tps")
            nc.tensor.matmul(yp[:], outwt[:], z[:], start=True,
                             stop=True)
            yf = smallp.tile([2, G], f32, tag="yf")
            nc.vector.tensor_scalar(yf[:], yp[:], outbt[:], None,
                                    ALU.add)
            nc.sync.dma_start(yout[:], yf[:])
    nc.finalize()
    return nc


def run(inputs, cfg, **run_kw):
    data, inv_cnt = host_prep(cfg, inputs["adj"], inputs["batch"])
    fl = prep_float_inputs(cfg, inputs)
    NC, NLOC, NSL = cfg["NC"], cfg["NLOC"], cfg["NSL"]
    in_maps = []
    for c in range(NC):
        m = dict(fl)
        m["inv_cnt"] = inv_cnt
        fto = np.zeros((cfg["FEAT"], NSL), np.float16)
        fto[:, :NLOC] = np.asarray(inputs["features"],
                                   np.float32)[c * NLOC:(c + 1) * NLOC].T
        m["featTown"] = fto
        m.update({k: data[c][k] for k in
                  ("g1", "g2", "dstv", "s3", "cmb", "wplace", "onehot")})
        in_maps.append(m)
    nc = build_program(cfg)
    from concourse.bass_utils import run_bass_kernel_spmd
    res = run_bass_kernel_spmd(nc, in_maps, list(range(NC)), **run_kw)
    y = np.asarray(res.results[0]["y"])
    return y.T.copy(), res


def kernel(**inputs):
    y, _ = run(inputs, make_cfg())
    return y
